# revision 56
# baseline (speedup 1.0000x reference)
"""Trainium2 Bass kernel for nn_Decoder_16054587752897.

Decoder block: banded additive (Bahdanau) attention + LN + FFN + LN +
3x (nearest-upsample-2x + conv1d k=7 + relu) + conv1d k=11 + sigmoid.

Sharding: pure data parallel - batch N=8, one batch element per NeuronCore.

v4 structure (from v3's 72us trace):
 - DMA ring discipline: only the two critical loads (x on the sync ring,
   the small weight blob on the scalar ring) are issued before the first
   compute; the big conv-weight blob, the f32 constants and the adense
   zero-fill are issued mid-band so their descriptors never sit ahead of
   the Xpb cast / q-k matmuls on a completion semaphore.
 - bh is folded into the k-projection (ones-row appended to Xpb, bh row
   appended to Wx), so no Scalar bias pass and no early wf32 dependency.
 - single ACT table set for the whole kernel (exp_and_others covers
   Exp/Tanh/Relu/Identity/Copy); LN rstd is computed on DVE with the
   int-bitcast Newton rsqrt, so the Sqrt set is never loaded and the
   sigmoid tail needs no table switch.
 - the Xw/XiT window transposes run before the band (PE is idle there)
   with their PSUM->SBUF copies on the otherwise-idle GpSimd engine.
 - adense uses a 192-wide row pitch; the banded-A scatter is 4 per-region
   DMAs and each chunk's gather is a contiguous-row read paired on the
   same ring as its scatter.
 - FFN runs chunk-pipelined: per 128-column chunk, the three FFN-1
   matmuls + relu feed the transposed FFN-2 accumulation immediately.
 - conv1/conv2 lhsT duplicate the output channels in M so the shifted
   replica rows land straight from PSUM; conv3's 4-fold replicas for the
   k=11 output conv are chunked SBUF DMAs; the sigmoid tail processes 4
   chunks per ACT op.
"""

import os
import sys

for _p in ("/opt/trn_rl_repo",):
    if _p not in sys.path:
        sys.path.insert(0, _p)

import numpy as np
from contextlib import ExitStack

import concourse.bass as bass
import concourse.bacc as bacc
import concourse.mybir as mybir
import concourse.tile as tile
from concourse.bass_utils import run_bass_kernel_spmd

F32 = mybir.dt.float32
I32 = mybir.dt.int32
BF16 = mybir.dt.bfloat16
AF = mybir.ActivationFunctionType
ALU = mybir.AluOpType
AX = mybir.AxisListType

L = 512
C = 96
EPS_ATTN = 1e-6
EPS_LN = 1e-5
RSQRT_MAGIC = 0x5F3759DF


# ----------------------------------------------------------------------------
# host-side constant prep (weight-only transforms)
# ----------------------------------------------------------------------------

def _host_prep(inp):
    f = lambda k: np.ascontiguousarray(np.asarray(inp[k], np.float32))
    p = {}
    p['Wt'] = f('Wt')                       # [96, 32] lhsT for q
    # bh folded into k: ones-row in Xpb meets the bh row of Wx97
    p['Wx97'] = np.vstack([f('Wx'), f('bh')[None, :]])   # [97, 32]
    Wa = f('Wa')[:, 0]
    blockWa4 = np.zeros((128, 4), np.float32)
    for c in range(4):
        blockWa4[32 * c:32 * c + 32, c] = Wa
    p['blockWa4'] = blockWa4
    il = np.arange(128)[:, None, None]
    cc = np.arange(4)[None, :, None]
    oo = np.arange(64)[None, None, :]
    jj = cc * 128 + il + oo - 32
    p['bandmask'] = ((jj >= 0) & (jj < L)).astype(np.float32).reshape(128, 256)
    p['identb'] = np.eye(128, dtype=np.float32)
    # broadcast-rows blob: LN0 g|b, LN1 g|b, ffn b1 (each 96 wide)
    grows = np.zeros((1, 480), np.float32)
    grows[0, 0:96] = f('ln0_g')
    grows[0, 96:192] = f('ln0_b')
    grows[0, 192:288] = f('ln1_g')
    grows[0, 288:384] = f('ln1_b')
    grows[0, 384:480] = f('ff_b1')
    p['grows'] = grows
    p['one1_128'] = np.ones((1, 128), np.float32)
    p['w0T'] = np.ascontiguousarray(f('ff_w0').T)                # [96, 384]
    p['fb0'] = np.ascontiguousarray(f('ff_b0').reshape(3, 128).T)  # [128, 3]
    # w1T [128, 3*96]: cols s*96+c = ff_w1[c, s*128+h]
    w1 = f('ff_w1')                                              # [96, 384]
    w1T = np.zeros((128, 288), np.float32)
    for s in range(3):
        w1T[:, s * 96:(s + 1) * 96] = w1[:, s * 128:(s + 1) * 128].T
    p['w1T'] = w1T

    def eo(w):
        # w: [co, ci, 7] -> even/odd tap-summed lhsT banks [ci, 4*co]
        We = np.stack([w[:, :, 0], w[:, :, 1] + w[:, :, 2],
                       w[:, :, 3] + w[:, :, 4], w[:, :, 5] + w[:, :, 6]])
        Wo = np.stack([w[:, :, 0] + w[:, :, 1], w[:, :, 2] + w[:, :, 3],
                       w[:, :, 4] + w[:, :, 5], w[:, :, 6]])
        co, ci = w.shape[0], w.shape[1]
        pack = lambda Ws: np.ascontiguousarray(
            Ws.transpose(2, 0, 1).reshape(ci, 4 * co))
        return pack(We), pack(Wo)

    W1e, W1o = eo(f('up_w0'))             # [96, 256]
    W2e, W2o = eo(f('up_w1'))             # [64, 192]
    W3e, W3o = eo(f('up_w2'))             # [48, 128]

    def dup_m(W, ci, co):
        # [ci, 4*co] -> [ci, 4*(2co)]: each tap block duplicated in M so the
        # PSUM rows co:2co replicate rows 0:co (written as the shifted copy)
        out = np.zeros((ci, 8 * co), np.float32)
        for t in range(4):
            out[:, t * 2 * co:t * 2 * co + co] = W[:, t * co:(t + 1) * co]
            out[:, t * 2 * co + co:(t + 1) * 2 * co] = W[:, t * co:(t + 1) * co]
        return out
    p['W1ed'] = dup_m(W1e, 96, 64)   # [96, 512] taps of [96, 128]
    p['W1od'] = dup_m(W1o, 96, 64)

    def pack2(W, ci, co):
        # [ci, 4*co] tap-major -> [2*ci, 2*co]: rows tau'*ci+c_i,
        # pair p covers taps (2p, 2p+1)
        out = np.zeros((2 * ci, 2 * co), np.float32)
        for g in range(2):
            for tau in range(2):
                t = 2 * g + tau
                out[tau * ci:(tau + 1) * ci, g * co:(g + 1) * co] = \
                    W[:, t * co:(t + 1) * co]
        return out

    def pack2_dup112(W):
        # conv2: pair-packed K=128 (2 taps x 64ci); M=112 with the dup copy
        # at rows 64:112 so both PSUM write-outs sit at 32-aligned bases
        P = pack2(W, 64, 48)             # [128, 96] pairs of [128, 48]
        out = np.zeros((128, 224), np.float32)
        for g in range(2):
            out[:, g * 112:g * 112 + 48] = P[:, g * 48:(g + 1) * 48]
            out[:, g * 112 + 64:g * 112 + 112] = P[:, g * 48:(g + 1) * 48]
        return out
    p['W2ed'] = pack2_dup112(W2e)        # [128, 224] pairs of [128, 112]
    p['W2od'] = pack2_dup112(W2o)

    def pack2_pad128(W):
        # conv3: pair-packed with tau'=0 at K rows 0:48 and tau'=1 at rows
        # 64:112, matching h2q's (main, replica) row placement; rows
        # 48:64/112:128 are zero (h2q is fully zeroed at startup)
        P = pack2(W, 48, 32)             # [96, 64] pairs of [96, 32]
        out = np.zeros((128, 64), np.float32)
        for g in range(2):
            out[0:48, g * 32:(g + 1) * 32] = P[0:48, g * 32:(g + 1) * 32]
            out[64:112, g * 32:(g + 1) * 32] = P[48:96, g * 32:(g + 1) * 32]
        return out
    p['W3e2'] = pack2_pad128(W3e)        # [128, 64] pairs of [128, 32]
    p['W3o2'] = pack2_pad128(W3o)
    p['cb1d'] = np.tile(f('up_b0'), 2).reshape(128, 1)
    cb2q = np.zeros((128, 1), np.float32)
    cb2q[0:48, 0] = f('up_b1')
    cb2q[64:112, 0] = f('up_b1')
    p['cb2q'] = cb2q
    p['cb3'] = f('up_b2').reshape(32, 1)
    ow = f('out_w')[0]                    # (32, 11)
    Wog = np.zeros((128, 3), np.float32)
    for g in range(3):
        for r in range(4):
            t = 4 * g + r
            if t < 11:
                Wog[32 * r:32 * r + 32, g] = ow[:, t]
    p['Wog'] = Wog
    p['ob_half'] = float(f('out_b')[0]) / 2.0
    p['obh128'] = np.full((128, 1), p['ob_half'], np.float32)

    # ---- pack into blobs ----
    packed = {}
    for blob, names in (('wf32', F32_PACK), ('wb16a', BF16A_PACK),
                        ('wb16b', BF16B_PACK)):
        width = sum(p[n].shape[1] for n in names)
        buf = np.zeros((128, width), np.float32)
        col = 0
        for n in names:
            a = p[n]
            buf[:a.shape[0], col:col + a.shape[1]] = a
            col += a.shape[1]
        packed[blob] = buf
    packed['shapes'] = {n: p[n].shape for n in
                        list(F32_PACK) + list(BF16A_PACK) + list(BF16B_PACK)}
    packed['ob_half'] = p['ob_half']
    packed['ln0_identity'] = bool(np.all(inp['ln0_g'] == 1.0)
                                  and np.all(inp['ln0_b'] == 0.0))
    packed['ln1_identity'] = bool(np.all(inp['ln1_g'] == 1.0)
                                  and np.all(inp['ln1_b'] == 0.0))
    packed['fb1_zero'] = bool(np.all(inp['ff_b1'] == 0.0))
    return packed


F32_PACK = ('fb0', 'cb1d', 'cb2q', 'cb3', 'obh128')
BF16A_PACK = ('Wt', 'Wx97', 'blockWa4', 'identb', 'bandmask',
              'one1_128', 'grows')
BF16B_PACK = ('w0T', 'w1T', 'W1ed', 'W1od', 'W2ed', 'W2od',
              'W3e2', 'W3o2', 'Wog')


# ----------------------------------------------------------------------------
# device kernel build
# ----------------------------------------------------------------------------

def _bcast_free(ap_full, offset_ap, counts):
    """Custom AP on the same tensor: dims [[pstep, 128]] + counts pairs."""
    pstep = ap_full.ap[0][0]
    return bass.AP(ap_full.tensor, offset_ap.offset,
                   [[pstep, ap_full.ap[0][1]]] + list(counts))


def _build(nc, tc, t_in, t_out, tp):
    x_ap = t_in.ap()          # [96, 512] fp32 in DRAM
    # banded-A scratch: 4 regions [128 rows (i), 192 cols (j window)]
    adense = nc.dram_tensor("adense", [4, 128, 192], BF16)
    RPITCH = 192
    RSTRIDE = 128 * RPITCH    # 24576 elements per region

    with ExitStack() as ctx:
        pw = ctx.enter_context(tc.tile_pool(name="weights", bufs=1))
        ps = ctx.enter_context(tc.tile_pool(name="seq", bufs=1))

        shapes = tp['shapes']
        wb16a = pw.tile(list(tp['wb16a'][1]), BF16, tag="wb16a")
        wf32 = pw.tile(list(tp['wf32'][1]), F32, tag="wf32")
        wb16b = pw.tile(list(tp['wb16b'][1]), BF16, tag="wb16b")
        w = {}
        for blob_tile, names in ((wf32, F32_PACK), (wb16a, BF16A_PACK),
                                 (wb16b, BF16B_PACK)):
            col = 0
            for n in names:
                r, cw = shapes[n]
                w[n] = blob_tile[0:r, col:col + cw]
                col += cw

        # ---------------- stage 0: input + weight loads ----------------
        # x arrives as bf16 and lands directly in the padded Xpb tile (no
        # cast op); critical loads lead each ring, bulk blobs drain behind
        # them during q/k + the band, and the adense zero-fill rides last
        # on the scalar ring so both rings are clean again by scatter time.
        Xpb = ps.tile([97, 576], BF16, tag="Xpb")
        nc.gpsimd.memset(Xpb[0:96, 0:32], 0.0)
        nc.gpsimd.memset(Xpb[0:96, 544:576], 0.0)
        nc.gpsimd.memset(Xpb[96:97, 0:576], 1.0)
        nc.sync.dma_start(Xpb[0:96, 32:544], x_ap)
        nc.scalar.dma_start(wb16a[:], tp['wb16a'][0].ap())
        nc.sync.dma_start(wf32[:], tp['wf32'][0].ap())
        nc.scalar.dma_start(wb16b[:], tp['wb16b'][0].ap())
        zz = ps.tile([128, 768], BF16, tag="zz")
        nc.gpsimd.memset(zz[:], 0.0)
        nc.scalar.dma_start(bass.AP(adense, 0, [[768, 128], [1, 768]]), zz[:])

        # warm the exp_and_others ACT table set (Exp anchors the set;
        # Tanh/Relu/Identity/Copy ride along, so this is the only load)
        warm = ps.tile([1, 2], F32, tag="warm")
        nc.gpsimd.memset(warm[:], 0.0)
        nc.scalar.activation(warm[:], warm[:], AF.Exp)
        nc.scalar.activation(warm[:], warm[:], AF.Tanh)
        eps128 = ps.tile([128, 1], F32, tag="eps128")
        nc.gpsimd.memset(eps128[:], EPS_LN)

        # ---------------- attention: q/k (+halo wings via matmul) -------
        Q4 = ps.tile([128, 128], BF16, tag="Q4")
        K4pad = ps.tile([128, 192], BF16, tag="K4pad")
        nc.gpsimd.memset(K4pad[96:128, 160:192], 0.0)

        with tc.tile_pool(name="qk_ps", bufs=3, space="PSUM") as pp:
            k_ps = pp.tile([128, 128], F32, tag="qk")
            for c in range(4):
                nc.tensor.matmul(k_ps[32 * c:32 * c + 32, :], w['Wx97'],
                                 Xpb[0:97, 32 + c * 128:32 + (c + 1) * 128],
                                 tile_position=(0, 32 * c))
            nc.vector.tensor_copy(K4pad[:, 32:160], k_ps[:])
            # halo wings: left = k of previous chunk's last 32 cols,
            # right = k of next chunk's first 32 cols
            wing_ps = pp.tile([128, 64], F32, tag="qk")
            for c in range(4):
                nc.tensor.matmul(wing_ps[32 * c:32 * c + 32, 0:32], w['Wx97'],
                                 Xpb[0:97, c * 128:c * 128 + 32],
                                 tile_position=(0, 32 * c))
            for c in range(3):
                nc.tensor.matmul(wing_ps[32 * c:32 * c + 32, 32:64], w['Wx97'],
                                 Xpb[0:97, 32 + (c + 1) * 128:
                                     64 + (c + 1) * 128],
                                 tile_position=(0, 32 * c))
            nc.scalar.copy(K4pad[:, 0:32], wing_ps[:, 0:32])
            nc.scalar.copy(K4pad[0:96, 160:192], wing_ps[0:96, 32:64])
            q_ps = pp.tile([128, 128], F32, tag="qk")
            for c in range(4):
                nc.tensor.matmul(q_ps[32 * c:32 * c + 32, :], w['Wt'],
                                 Xpb[0:96, 32 + c * 128:32 + (c + 1) * 128],
                                 tile_position=(0, 32 * c))
            nc.vector.tensor_copy(Q4[:], q_ps[:])

        # LN gamma/beta (+ffn b1) broadcast rows; skipped entirely when the
        # affine is identity and ff_b1 is zero (host-checked)
        need_gb = not (tp['ln0_identity'] and tp['ln1_identity']
                       and tp['fb1_zero'])
        if need_gb:
            GBb = ps.tile([128, 480], BF16, tag="GBb")
            with tc.tile_pool(name="gb_ps", bufs=1, space="PSUM") as gbp:
                gb_ps = gbp.tile([128, 480], F32, tag="gb")
                nc.tensor.matmul(gb_ps[:], w['one1_128'], w['grows'])
                nc.vector.tensor_copy(GBb[:], gb_ps[:])
            Gb0, Bb0 = GBb[:, 0:96], GBb[:, 96:192]
            Gb1, Bb1 = GBb[:, 192:288], GBb[:, 288:384]
            Fb = GBb[:, 384:480]
        else:
            Gb0 = Bb0 = Gb1 = Bb1 = Fb = None

        # ---------------- attention: band logits ----------------
        # The Xw/XiT window transposes for AV are interleaved into the band:
        # group g's add/tanh/E-matmuls are followed by chunk g's transposes
        # (PE idle gaps) and copies (DVE slack behind the serial tanh chain).
        GO = 16  # offsets per group
        EXb = ps.tile([128, 256], BF16, tag="EXb")
        EXf = ps.tile([128, 256], BF16, tag="EXf")
        Xw = []
        XiT = ps.tile([128, 384], BF16, tag="XiT")

        with ExitStack() as ectx:
            pa_arg = ectx.enter_context(tc.tile_pool(name="arg_sb", bufs=2))
            pa_tan = ectx.enter_context(tc.tile_pool(name="tan_sb", bufs=3))
            pe = ectx.enter_context(tc.tile_pool(name="e_ps", bufs=1,
                                                 space="PSUM"))
            xt = ectx.enter_context(tc.tile_pool(name="xw_ps", bufs=2,
                                                 space="PSUM"))
            xt2 = ectx.enter_context(tc.tile_pool(name="xi_ps", bufs=2,
                                                  space="PSUM"))
            E_ps = pe.tile([128, 256], F32, tag="E")
            # first group halved so the serial tanh chain starts ~0.6us
            # earlier (the add is the only thing ahead of it)
            GROUPS = ((0, 8), (8, 8), (16, 16), (32, 16), (48, 16))
            for g, (o0, go) in enumerate(GROUPS):
                Targ = pa_arg.tile([128, go * 128], BF16, tag=f"Targ{go}")
                q_b = _bcast_free(Q4[:], Q4[:], [[0, go], [1, 128]])
                k_b = _bcast_free(K4pad[:], K4pad[:, o0:192], [[1, go], [1, 128]])
                nc.vector.tensor_add(
                    Targ[:].rearrange("p (o i) -> p o i", o=go), q_b, k_b)
                Ttan = pa_tan.tile([128, go * 128], BF16, tag=f"Ttan{go}")
                nc.scalar.activation(Ttan[:], Targ[:], AF.Tanh)
                for oi in range(go):
                    o = o0 + oi
                    nc.tensor.matmul(
                        E_ps[:].rearrange("p (c o) -> p c o", o=64)[:, :, o],
                        Ttan[:, oi * 128:(oi + 1) * 128], w['blockWa4'])
                if g == 2:
                    # o 0:32 of every chunk is complete: exp+mask+scatter
                    # the first half of the band under the remaining tanhs
                    # (rings and DVE are idle; one 400ns wedge on Scalar)
                    h1 = EXf[:].rearrange("p (c o) -> p c o", o=64)[:, :, 0:32]
                    nc.scalar.activation(
                        h1, E_ps[:].rearrange("p (c o) -> p c o", o=64)[:, :, 0:32],
                        AF.Exp)
                    mh1 = EXb[:].rearrange("p (c o) -> p c o", o=64)[:, :, 0:32]
                    nc.vector.tensor_mul(
                        mh1, h1,
                        w['bandmask'].rearrange("p (c o) -> p c o", o=64)[:, :, 0:32])
                    exb_ap = EXb[:]
                    nc.sync.dma_start(
                        bass.AP(adense, 0,
                                [[RPITCH + 1, 128], [2 * RSTRIDE, 2], [1, 32]]),
                        bass.AP(exb_ap.tensor, exb_ap.offset,
                                [[256, 128], [128, 2], [1, 32]]))
                    nc.scalar.dma_start(
                        bass.AP(adense, RSTRIDE,
                                [[RPITCH + 1, 128], [2 * RSTRIDE, 2], [1, 32]]),
                        bass.AP(exb_ap.tensor, exb_ap.offset + 64,
                                [[256, 128], [128, 2], [1, 32]]))
                if g == 0:
                    continue
                c = g - 1
                # Xw windows split as [j_loc 64:192] (cols 0:96, full 128
                # rows) + [j_loc 0:64] (cols 96:192, rows 0:64) to match the
                # XBAR-transposed At pieces
                x_ps = xt.tile([128, 192], BF16, tag="x")
                nc.tensor.transpose(x_ps[:, 0:96],
                                    Xpb[0:96, c * 128 + 64:c * 128 + 192],
                                    w['identb'][0:96, 0:96])
                nc.tensor.transpose(x_ps[0:64, 96:192],
                                    Xpb[0:96, c * 128:c * 128 + 64],
                                    w['identb'][0:96, 0:96])
                xw = ps.tile([128, 192], BF16, tag=f"Xw{c}")
                nc.vector.tensor_copy(xw[:, 0:96], x_ps[:, 0:96])
                nc.vector.tensor_copy(xw[0:64, 96:192], x_ps[0:64, 96:192])
                Xw.append(xw)
                xi_ps = xt2.tile([128, 96], BF16, tag="xi")
                nc.tensor.transpose(xi_ps[:],
                                    Xpb[0:96, 32 + c * 128:32 + (c + 1) * 128],
                                    w['identb'][0:96, 0:96])
                nc.vector.tensor_copy(XiT[:, c * 96:(c + 1) * 96], xi_ps[:])
                if g == 1:
                    # conv tile pads (consumed from LN1 onwards)
                    h0 = ps.tile([96, 516], BF16, tag="h0")
                    nc.gpsimd.memset(h0[:, 0:2], 0.0)
                    nc.gpsimd.memset(h0[:, 514:516], 0.0)
                    h1rep = ps.tile([128, 1028], BF16, tag="h1rep")
                    nc.gpsimd.memset(h1rep[:, 0:2], 0.0)
                    nc.gpsimd.memset(h1rep[:, 1024:1028], 0.0)
                    h2q = ps.tile([128, 2052], BF16, tag="h2q")
                    nc.gpsimd.memset(h2q[:], 0.0)
                    h3rep = ps.tile([128, 4112], BF16, tag="h3rep")
                    nc.gpsimd.memset(h3rep[:, 0:8], 0.0)
                    nc.gpsimd.memset(h3rep[:, 4104:4112], 0.0)
            # masked unnormalized weights, bf16 end to end (second half;
            # the first half went out mid-band)
            h2 = EXf[:].rearrange("p (c o) -> p c o", o=64)[:, :, 32:64]
            nc.scalar.activation(
                h2, E_ps[:].rearrange("p (c o) -> p c o", o=64)[:, :, 32:64],
                AF.Exp)
            nc.vector.tensor_mul(
                EXb[:].rearrange("p (c o) -> p c o", o=64)[:, :, 32:64], h2,
                w['bandmask'].rearrange("p (c o) -> p c o", o=64)[:, :, 32:64])


        exb_ap = EXb[:]
        nc.sync.dma_start(
            bass.AP(adense, 32,
                    [[RPITCH + 1, 128], [2 * RSTRIDE, 2], [1, 32]]),
            bass.AP(exb_ap.tensor, exb_ap.offset + 32,
                    [[256, 128], [128, 2], [1, 32]]))
        nc.scalar.dma_start(
            bass.AP(adense, RSTRIDE + 32,
                    [[RPITCH + 1, 128], [2 * RSTRIDE, 2], [1, 32]]),
            bass.AP(exb_ap.tensor, exb_ap.offset + 96,
                    [[256, 128], [128, 2], [1, 32]]))
        S4 = ps.tile([128, 4], F32, tag="S4")
        nc.vector.tensor_reduce(S4[:], EXb[:].rearrange("p (c o) -> p c o", o=64),
                                AX.X, ALU.add)
        nc.vector.tensor_scalar_add(S4[:], S4[:], EPS_ATTN)
        R4 = ps.tile([128, 4], F32, tag="R4")
        nc.vector.reciprocal(R4[:], S4[:])
        # pull the rsqrt table-set load into the scatter/gather latency
        # window; reading R4 anchors it after the softmax exp + row sums
        # (the scheduler hoists dep-free ops arbitrarily early)
        warmr = ps.tile([1, 2], F32, tag="warmr")
        nc.scalar.activation(warmr[:], R4[0:1, 0:2], AF.Abs_reciprocal_sqrt)

        # ---------------- attention: AV (v^T[i,ch]) + LN0 stats ----------
        vT_i = ps.tile([128, 384], BF16, tag="vTi")
        bns0 = ps.tile([128, 24], F32, tag="bns0")
        MV0 = ps.tile([128, 8], F32, tag="MV0")
        with ExitStack() as actx:
            pad = actx.enter_context(tc.tile_pool(name="ad_sb", bufs=4))
            pat = actx.enter_context(tc.tile_pool(name="at_sb", bufs=4))
            ptp = actx.enter_context(tc.tile_pool(name="at_ps", bufs=3,
                                                  space="PSUM"))
            pv = actx.enter_context(tc.tile_pool(name="v_ps", bufs=2,
                                                 space="PSUM"))
            for c in range(4):
                Ad = pad.tile([128, 192], BF16, tag="Ad")
                eng = nc.sync if c % 2 == 0 else nc.scalar
                eng.dma_start(Ad[:], bass.AP(adense, c * RSTRIDE,
                                             [[RPITCH, 128], [1, RPITCH]]))
                t_ps = ptp.tile([128, 256], BF16, tag="tp")
                nc.tensor.transpose(t_ps[:, 0:128], Ad[:, 64:192],
                                    w['identb'])
                nc.tensor.transpose(t_ps[0:64, 128:256], Ad[:, 0:64],
                                    w['identb'])
                At = pat.tile([128, 256], BF16, tag="At")
                nc.vector.tensor_copy(At[:, 0:128], t_ps[:, 0:128])
                nc.scalar.copy(At[0:64, 128:256], t_ps[0:64, 128:256])
                v_ps = pv.tile([128, 96], F32, tag="v")
                nc.tensor.matmul(v_ps[:], At[:, 0:128], Xw[c][:, 0:96],
                                 start=True, stop=False)
                nc.tensor.matmul(v_ps[:], At[0:64, 128:256],
                                 Xw[c][0:64, 96:192],
                                 start=False, stop=True)
                # v*R + x^T in one pass (R4 is per-partition here)
                nc.vector.scalar_tensor_tensor(
                    vT_i[:, c * 96:(c + 1) * 96], v_ps[:], R4[:, c:c + 1],
                    XiT[:, c * 96:(c + 1) * 96], ALU.mult, ALU.add)
                nc.vector.bn_stats(bns0[:, 6 * c:6 * c + 6],
                                   vT_i[:, c * 96:(c + 1) * 96])
                nc.vector.bn_aggr(MV0[:, 2 * c:2 * c + 2],
                                  bns0[:, 6 * c:6 * c + 6])

        # ---------------- LN tails (i-layout, ACT rsqrt) ----------------
        def rstd_act(MV, tag):
            """rstd[128,4] = Rsqrt(var+eps) in one ACT op; the
            reciprocal_sqrt_and_small table set is pre-warmed during the
            attention scatter/gather window and also covers the FFN/conv
            Relus, so no load lands on the LN critical path."""
            rstd = ps.tile([128, 4], F32, tag=f"rstd{tag}", name=f"rstd{tag}")
            mv_ap = MV[:]
            var_ap = bass.AP(mv_ap.tensor, mv_ap.offset + 1, [[8, 128], [2, 4]])
            nc.scalar.activation(rstd[:], var_ap, AF.Abs_reciprocal_sqrt,
                                 bias=eps128[:])
            return rstd

        rstd_aps = {}

        def ln_i(MV, src, Gb, Bb, identity, sink, tag, keep=False):
            rstd = rstd_act(MV, tag)
            rstd_aps[tag] = rstd
            with tc.tile_pool(name=f"ln{tag}_sb", bufs=2) as ly:
                for c in range(4):
                    def final_tile():
                        if keep:
                            return ps.tile([128, 96], BF16, tag=f"yk{tag}{c}",
                                           name=f"yk{tag}{c}")
                        return ly.tile([128, 96], BF16, tag="yf",
                                       name=f"yf{tag}")
                    if identity:
                        y0 = final_tile()
                    else:
                        y0 = ly.tile([128, 96], BF16, tag="y0",
                                     name=f"y0{tag}")
                    eng = nc.vector if c % 2 == 0 else nc.gpsimd
                    eng.tensor_scalar(y0[:], src[:, c * 96:(c + 1) * 96],
                                      MV[:, 2 * c:2 * c + 1],
                                      rstd[:, c:c + 1],
                                      ALU.subtract, ALU.mult)
                    if identity:
                        sink(c, y0)
                        continue
                    y1 = ly.tile([128, 96], BF16, tag="y1", name=f"y1{tag}")
                    nc.vector.tensor_mul(y1[:], y0[:], Gb)
                    y2 = final_tile()
                    nc.vector.tensor_add(y2[:], y1[:], Bb)
                    sink(c, y2)

        # LN0 output chunks stay live (x2 residual for FFN-2) - no i-layout
        # copy needed; x2b is the transposed view for the FFN-1 rhs
        x2c = []
        x2b = ps.tile([96, 512], BF16, tag="x2b")
        with tc.tile_pool(name="ln0_ps", bufs=2, space="PSUM") as lp0:
            def sink0(c, y2):
                x2c.append(y2)
                xp_ps = lp0.tile([96, 128], BF16, tag="xp", name="xp0")
                nc.tensor.transpose(xp_ps[:], y2[:], w['identb'])
                nc.scalar.copy(x2b[:, c * 128:(c + 1) * 128], xp_ps[:])
            ln_i(MV0, vT_i, Gb0, Bb0, tp['ln0_identity'], sink0, "0",
                 keep=True)

        # ---------------- FFN (chunk-pipelined) ----------------
        x4T = ps.tile([128, 384], BF16, tag="x4T")
        bns1 = ps.tile([128, 24], F32, tag="bns1")
        MV1 = ps.tile([128, 8], F32, tag="MV1")
        with ExitStack() as fctx:
            fp = fctx.enter_context(tc.tile_pool(name="ffn_sb", bufs=1))
            fpp = fctx.enter_context(tc.tile_pool(name="ffn_ps", bufs=1,
                                                  space="PSUM"))
            f2 = fctx.enter_context(tc.tile_pool(name="f2_sb", bufs=2))
            f2p = fctx.enter_context(tc.tile_pool(name="f2_ps", bufs=2,
                                                  space="PSUM"))
            h_ps = [fpp.tile([128, 512], F32, tag=f"h{s}", name=f"h{s}")
                    for s in range(3)]
            Hr = [fp.tile([128, 512], BF16, tag=f"hr{s}", name=f"hr{s}")
                  for s in range(3)]
            for c in range(4):
                sl = slice(c * 128, (c + 1) * 128)
                for s in range(3):
                    nc.tensor.matmul(h_ps[s][:, sl],
                                     w['w0T'][:, s * 128:(s + 1) * 128],
                                     x2b[:, sl])
                    if (c + s) % 2 == 0:
                        nc.scalar.activation(Hr[s][:, sl], h_ps[s][:, sl],
                                             AF.Relu, bias=w['fb0'][:, s:s + 1])
                    else:
                        nc.vector.tensor_scalar(Hr[s][:, sl], h_ps[s][:, sl],
                                                w['fb0'][:, s:s + 1], 0.0,
                                                ALU.add, ALU.max)
                # FFN-2 transposed: x3^T[i, ch] for this chunk
                x3_ps = f2p.tile([128, 96], F32, tag="x3T", name="x3T")
                for s in range(3):
                    nc.tensor.matmul(x3_ps[:], Hr[s][:, sl],
                                     w['w1T'][:, s * 96:(s + 1) * 96],
                                     start=(s == 0), stop=(s == 2))
                if tp['fb1_zero']:
                    nc.vector.tensor_add(x4T[:, c * 96:(c + 1) * 96],
                                         x3_ps[:], x2c[c][:])
                else:
                    t0 = f2.tile([128, 96], BF16, tag="t0", name="t0")
                    nc.vector.tensor_add(t0[:], x3_ps[:], Fb)
                    nc.vector.tensor_add(x4T[:, c * 96:(c + 1) * 96], t0[:],
                                         x2c[c][:])
                nc.vector.bn_stats(bns1[:, 6 * c:6 * c + 6],
                                   x4T[:, c * 96:(c + 1) * 96])
                nc.vector.bn_aggr(MV1[:, 2 * c:2 * c + 2],
                                  bns1[:, 6 * c:6 * c + 6])

        # ---------------- LN1 (i-layout) -> h0 ----------------
        with tc.tile_pool(name="ln1_ps", bufs=2, space="PSUM") as lp1:
            def sink1(c, y2):
                xp_ps = lp1.tile([96, 128], BF16, tag="xp", name="xp1")
                nc.tensor.transpose(xp_ps[:], y2[:], w['identb'])
                if c % 2 == 0:
                    nc.vector.tensor_copy(h0[:, 2 + c * 128:2 + (c + 1) * 128],
                                          xp_ps[:])
                else:
                    nc.scalar.copy(h0[:, 2 + c * 128:2 + (c + 1) * 128],
                                   xp_ps[:])
            ln_i(MV1, x4T, Gb1, Bb1, tp['ln1_identity'], sink1, "1")

        # ---------------- conv stack ----------------
        def strided_dst(hout, p0, cout, col0, n=512):
            hap = hout[:]
            return bass.AP(hap.tensor, hout[p0:p0 + 1, col0:col0 + 1].offset,
                           [[hap.ap[0][0], cout], [2, n]])

        # epilogue column pieces: the first piece covers everything the next
        # conv's k=0 matmuls read (dst cols <= 516+par), so they can start
        # while the second piece is still being written
        EPI_SPLIT = ((0, 258), (258, 254))

        # conv1: K=96, 4 taps x 2 phases, M=128 (dup) -> h1 + shifted replica
        with tc.tile_pool(name="c1_ps", bufs=1, space="PSUM") as cp1:
            zs1 = {}
            for par, Wp in ((0, 'W1ed'), (1, 'W1od')):
                z = cp1.tile([128, 512], F32, tag=f"z1{par}", name=f"z1{par}")
                zs1[par] = z
                for tau in range(4):
                    nc.tensor.matmul(z[:], w[Wp][:, tau * 128:(tau + 1) * 128],
                                     h0[:, par + tau:par + tau + 512],
                                     start=(tau == 0), stop=(tau == 3))
            for par in (0, 1):
                z = zs1[par]
                if par == 0:
                    nc.scalar.activation(strided_dst(h1rep, 0, 64, 2 + par),
                                         z[0:64, :], AF.Relu,
                                         bias=w['cb1d'][0:64, :])
                    nc.vector.tensor_scalar(
                        strided_dst(h1rep, 64, 64, 1 + par), z[64:128, :],
                        w['cb1d'][64:128, :], 0.0, ALU.add, ALU.max)
                else:
                    nc.vector.tensor_scalar(
                        strided_dst(h1rep, 0, 64, 2 + par), z[0:64, :],
                        w['cb1d'][0:64, :], 0.0, ALU.add, ALU.max)
                    nc.scalar.activation(strided_dst(h1rep, 64, 64, 1 + par),
                                         z[64:128, :], AF.Relu,
                                         bias=w['cb1d'][64:128, :])

        # conv2: K=128 (2 taps packed), M=112 (dup at rows 64:112) ->
        # h2q rows 0:48 main, rows 64:112 shifted replica
        with tc.tile_pool(name="c2_ps", bufs=1, space="PSUM") as cp2:
            psums = {}
            for par in (0, 1):
                for k in range(2):
                    psums[(par, k)] = cp2.tile([112, 512], F32,
                                               tag=f"z2{par}{k}",
                                               name=f"z2_{par}_{k}")
            for par, Wp in ((0, 'W2ed'), (1, 'W2od')):
                for pq in range(2):
                    for k in range(2):
                        nc.tensor.matmul(
                            psums[(par, k)][:],
                            w[Wp][:, pq * 112:(pq + 1) * 112],
                            h1rep[:, par + 2 * pq + k * 512:
                                  par + 2 * pq + k * 512 + 512],
                            start=(pq == 0), stop=(pq == 1))
            for par in (0, 1):
                for k in range(2):
                    z = psums[(par, k)]
                    c0 = 2 + 2 * k * 512 + par
                    if (par + k) % 2 == 0:
                        nc.scalar.activation(strided_dst(h2q, 0, 48, c0),
                                             z[0:48, :], AF.Relu,
                                             bias=w['cb2q'][0:48, :])
                        nc.vector.tensor_scalar(
                            strided_dst(h2q, 64, 48, c0 - 1), z[64:112, :],
                            w['cb2q'][64:112, :], 0.0, ALU.add, ALU.max)
                    else:
                        nc.vector.tensor_scalar(
                            strided_dst(h2q, 0, 48, c0), z[0:48, :],
                            w['cb2q'][0:48, :], 0.0, ALU.add, ALU.max)
                        nc.scalar.activation(strided_dst(h2q, 64, 48, c0 - 1),
                                             z[64:112, :], AF.Relu,
                                             bias=w['cb2q'][64:112, :])

        # pull the tanh/exp table-set reload into the conv2/conv3 matmul
        # window so the sigmoid tail doesn't pay for it; reading the first
        # h2q column anchors it behind conv2's first epilogue only
        warmt = ps.tile([1, 2], BF16, tag="warmt")
        nc.scalar.activation(warmt[:], h2q[0:1, 2:4], AF.Tanh)

        # conv3: K=96 (2 taps packed), 2 pairs x 2 phases x 4 chunks;
        # 4-fold replicas for the output conv built per chunk so DMA
        # receipts hide under later chunks' compute
        with tc.tile_pool(name="c3_ps", bufs=1, space="PSUM") as cp3:
            psums = {}
            for par in (0, 1):
                for k in range(4):
                    psums[(par, k)] = cp3.tile([32, 512], F32,
                                               tag=f"z3{par}{k}",
                                               name=f"z3_{par}_{k}")
            for par, Wp in ((0, 'W3e2'), (1, 'W3o2')):
                for pq in range(2):
                    for k in range(4):
                        nc.tensor.matmul(
                            psums[(par, k)][:],
                            w[Wp][:, pq * 32:(pq + 1) * 32],
                            h2q[:, par + 2 * pq + k * 512:
                                par + 2 * pq + k * 512 + 512],
                            start=(pq == 0), stop=(pq == 1))
            for k in range(4):
                for par in (0, 1):
                    z = psums[(par, k)]
                    c0 = 8 + 2 * k * 512 + par
                    if par == 0:
                        nc.scalar.activation(strided_dst(h3rep, 0, 32, c0),
                                             z[:], AF.Relu, bias=w['cb3'])
                    else:
                        nc.vector.tensor_scalar(strided_dst(h3rep, 0, 32, c0),
                                                z[:], w['cb3'], 0.0,
                                                ALU.add, ALU.max)
                C0 = 8 + 2 * k * 512
                hi = 4112 if k == 3 else C0 + 1024
                for r in (1, 2, 3):
                    eng = (nc.sync, nc.scalar, nc.sync)[r - 1] if k % 2 == 0 \
                        else (nc.scalar, nc.sync, nc.scalar)[r - 1]
                    eng.dma_start(h3rep[32 * r:32 * r + 32, C0 - r:hi - r],
                                  h3rep[0:32, C0:hi])


        # ---------------- output conv (col-tiled) + sigmoid ----------------
        with ExitStack() as octx:
            yo = octx.enter_context(tc.tile_pool(name="yo_sb", bufs=2))
            yp = octx.enter_context(tc.tile_pool(name="yo_ps", bufs=2,
                                                 space="PSUM"))
            for b in range(2):
                y_ps = yp.tile([128, 512], F32, tag="yo", name="y_ps")
                for j in range(4):
                    k = 4 * b + j
                    for g in range(3):
                        rhs = h3rep[:, k * 512 + 4 * g + 3:
                                    k * 512 + 4 * g + 3 + 512]
                        nc.tensor.matmul(y_ps[32 * j:32 * j + 1, :],
                                         w['Wog'][:, g:g + 1], rhs,
                                         start=(g == 0), stop=(g == 2),
                                         tile_position=(0, 32 * j))
                ysig = yo.tile([128, 512], F32, tag="ysig", name="ysig")
                nc.scalar.activation(ysig[0:97, :], y_ps[0:97, :], AF.Tanh,
                                     bias=w['obh128'][0:97, :], scale=0.5)
                out4 = yo.tile([128, 512], F32, tag="out4", name="out4")
                nc.vector.tensor_scalar(out4[0:97, :], ysig[0:97, :],
                                        0.5, 0.5, ALU.mult, ALU.add)
                o_ap = out4[:]
                src = bass.AP(o_ap.tensor, o_ap.offset,
                              [[512 * 32, 4], [1, 512]])
                dst = bass.AP(t_out, b * 2048, [[512, 4], [1, 512]])
                eng = nc.sync if b == 0 else nc.scalar
                eng.dma_start(dst, src)


# ----------------------------------------------------------------------------
# public entry point
# ----------------------------------------------------------------------------

def build_module(p):
    import ml_dtypes
    nc = bacc.Bacc("TRN2", target_bir_lowering=False, debug=False)
    t_in = nc.dram_tensor("x", [C, L], BF16, kind="ExternalInput")
    t_out = nc.dram_tensor("out", [1, 4096], F32, kind="ExternalOutput")
    tp = {}
    # weight blobs ride inside the NEFF as Const tensors: the runtime DMAs
    # them to HBM at model-load time, so no staging transfer or completion
    # wait lands in the execution window
    for blob, dt, npdt in (('wf32', F32, np.float32),
                           ('wb16a', BF16, ml_dtypes.bfloat16),
                           ('wb16b', BF16, ml_dtypes.bfloat16)):
        data = np.ascontiguousarray(p[blob].astype(npdt))
        tp[blob] = (nc.inline_tensor(data, name=blob), p[blob].shape, dt)
    tp['shapes'] = p['shapes']
    tp['ob_half'] = p['ob_half']
    tp['ln0_identity'] = p['ln0_identity']
    tp['ln1_identity'] = p['ln1_identity']
    tp['fb1_zero'] = p['fb1_zero']
    with tile.TileContext(nc) as tc:
        _build(nc, tc, t_in, t_out, tp)
    nc.compile()
    return nc


def kernel(**inputs):
    # The neuron compile cache keys on the HLO signature only (it does not
    # hash the embedded bass program), so a stale entry from a different
    # kernel revision with identical I/O shapes would silently load the
    # wrong NEFF. Purge unless the cache was stamped by this exact source.
    import hashlib
    import shutil
    me = hashlib.sha256(open(__file__, 'rb').read()).hexdigest()
    for cdir in ('/root/.neuron-compile-cache', '/var/tmp/neuron-compile-cache'):
        marker = os.path.join(cdir, '.kernel_src_hash')
        try:
            if open(marker).read() == me:
                continue
        except OSError:
            pass
        shutil.rmtree(cdir, ignore_errors=True)
        try:
            os.makedirs(cdir, exist_ok=True)
            with open(marker, 'w') as fh:
                fh.write(me)
        except OSError:
            pass

    x = np.asarray(inputs['x'], np.float32)          # (8, 96, 512)
    N = x.shape[0]
    p = _host_prep(inputs)
    nc = build_module(p)
    import ml_dtypes
    in_maps = [{'x': np.ascontiguousarray(x[n].astype(ml_dtypes.bfloat16))}
               for n in range(N)]
    res = run_bass_kernel_spmd(nc, in_maps, core_ids=list(range(N)))
    global LAST_RESULTS
    LAST_RESULTS = res
    out = np.stack([res.results[n]['out'] for n in range(N)], axis=0)
    return out.astype(np.float32)


LAST_RESULTS = None


if __name__ == '__main__':
    print("kernel.py loaded OK")


# revision 57
# speedup vs baseline: 1.0956x; 1.0956x over previous
"""Trainium2 Bass kernel for nn_Decoder_16054587752897.

Decoder block: banded additive (Bahdanau) attention + LN + FFN + LN +
3x (nearest-upsample-2x + conv1d k=7 + relu) + conv1d k=11 + sigmoid.

Sharding: pure data parallel - batch N=8, one batch element per NeuronCore.

v4 structure (from v3's 72us trace):
 - DMA ring discipline: only the two critical loads (x on the sync ring,
   the small weight blob on the scalar ring) are issued before the first
   compute; the big conv-weight blob, the f32 constants and the adense
   zero-fill are issued mid-band so their descriptors never sit ahead of
   the Xpb cast / q-k matmuls on a completion semaphore.
 - bh is folded into the k-projection (ones-row appended to Xpb, bh row
   appended to Wx), so no Scalar bias pass and no early wf32 dependency.
 - single ACT table set for the whole kernel (exp_and_others covers
   Exp/Tanh/Relu/Identity/Copy); LN rstd is computed on DVE with the
   int-bitcast Newton rsqrt, so the Sqrt set is never loaded and the
   sigmoid tail needs no table switch.
 - the Xw/XiT window transposes run before the band (PE is idle there)
   with their PSUM->SBUF copies on the otherwise-idle GpSimd engine.
 - adense uses a 192-wide row pitch; the banded-A scatter is 4 per-region
   DMAs and each chunk's gather is a contiguous-row read paired on the
   same ring as its scatter.
 - FFN runs chunk-pipelined: per 128-column chunk, the three FFN-1
   matmuls + relu feed the transposed FFN-2 accumulation immediately.
 - conv1/conv2 lhsT duplicate the output channels in M so the shifted
   replica rows land straight from PSUM; conv3's 4-fold replicas for the
   k=11 output conv are chunked SBUF DMAs; the sigmoid tail processes 4
   chunks per ACT op.
"""

import os
import sys

for _p in ("/opt/trn_rl_repo",):
    if _p not in sys.path:
        sys.path.insert(0, _p)

import numpy as np
from contextlib import ExitStack

import concourse.bass as bass
import concourse.bacc as bacc
import concourse.mybir as mybir
import concourse.tile as tile
from concourse.bass_utils import run_bass_kernel_spmd

F32 = mybir.dt.float32
I32 = mybir.dt.int32
BF16 = mybir.dt.bfloat16
AF = mybir.ActivationFunctionType
ALU = mybir.AluOpType
AX = mybir.AxisListType

L = 512
C = 96
EPS_ATTN = 1e-6
EPS_LN = 1e-5
RSQRT_MAGIC = 0x5F3759DF


# ----------------------------------------------------------------------------
# host-side constant prep (weight-only transforms)
# ----------------------------------------------------------------------------

def _host_prep(inp):
    f = lambda k: np.ascontiguousarray(np.asarray(inp[k], np.float32))
    p = {}
    p['Wt'] = f('Wt')                       # [96, 32] lhsT for q
    # bh folded into k: ones-row in Xpb meets the bh row of Wx97
    p['Wx97'] = np.vstack([f('Wx'), f('bh')[None, :]])   # [97, 32]
    Wa = f('Wa')[:, 0]
    blockWa4 = np.zeros((128, 4), np.float32)
    for c in range(4):
        blockWa4[32 * c:32 * c + 32, c] = Wa
    p['blockWa4'] = blockWa4
    il = np.arange(128)[:, None, None]
    cc = np.arange(4)[None, :, None]
    oo = np.arange(64)[None, None, :]
    jj = cc * 128 + il + oo - 32
    p['bandmask'] = ((jj >= 0) & (jj < L)).astype(np.float32).reshape(128, 256)
    p['identb'] = np.eye(128, dtype=np.float32)
    # broadcast-rows blob: LN0 g|b, LN1 g|b, ffn b1 (each 96 wide)
    grows = np.zeros((1, 480), np.float32)
    grows[0, 0:96] = f('ln0_g')
    grows[0, 96:192] = f('ln0_b')
    grows[0, 192:288] = f('ln1_g')
    grows[0, 288:384] = f('ln1_b')
    grows[0, 384:480] = f('ff_b1')
    p['grows'] = grows
    p['one1_128'] = np.ones((1, 128), np.float32)
    p['w0T'] = np.ascontiguousarray(f('ff_w0').T)                # [96, 384]
    p['fb0'] = np.ascontiguousarray(f('ff_b0').reshape(3, 128).T)  # [128, 3]
    # w1T [128, 3*96]: cols s*96+c = ff_w1[c, s*128+h]
    w1 = f('ff_w1')                                              # [96, 384]
    w1T = np.zeros((128, 288), np.float32)
    for s in range(3):
        w1T[:, s * 96:(s + 1) * 96] = w1[:, s * 128:(s + 1) * 128].T
    p['w1T'] = w1T

    def eo(w):
        # w: [co, ci, 7] -> even/odd tap-summed lhsT banks [ci, 4*co]
        We = np.stack([w[:, :, 0], w[:, :, 1] + w[:, :, 2],
                       w[:, :, 3] + w[:, :, 4], w[:, :, 5] + w[:, :, 6]])
        Wo = np.stack([w[:, :, 0] + w[:, :, 1], w[:, :, 2] + w[:, :, 3],
                       w[:, :, 4] + w[:, :, 5], w[:, :, 6]])
        co, ci = w.shape[0], w.shape[1]
        pack = lambda Ws: np.ascontiguousarray(
            Ws.transpose(2, 0, 1).reshape(ci, 4 * co))
        return pack(We), pack(Wo)

    W1e, W1o = eo(f('up_w0'))             # [96, 256]
    W2e, W2o = eo(f('up_w1'))             # [64, 192]
    W3e, W3o = eo(f('up_w2'))             # [48, 128]

    def dup_m(W, ci, co):
        # [ci, 4*co] -> [ci, 4*(2co)]: each tap block duplicated in M so the
        # PSUM rows co:2co replicate rows 0:co (written as the shifted copy)
        out = np.zeros((ci, 8 * co), np.float32)
        for t in range(4):
            out[:, t * 2 * co:t * 2 * co + co] = W[:, t * co:(t + 1) * co]
            out[:, t * 2 * co + co:(t + 1) * 2 * co] = W[:, t * co:(t + 1) * co]
        return out
    p['W1ed'] = dup_m(W1e, 96, 64)   # [96, 512] taps of [96, 128]
    p['W1od'] = dup_m(W1o, 96, 64)

    def pack2(W, ci, co):
        # [ci, 4*co] tap-major -> [2*ci, 2*co]: rows tau'*ci+c_i,
        # pair p covers taps (2p, 2p+1)
        out = np.zeros((2 * ci, 2 * co), np.float32)
        for g in range(2):
            for tau in range(2):
                t = 2 * g + tau
                out[tau * ci:(tau + 1) * ci, g * co:(g + 1) * co] = \
                    W[:, t * co:(t + 1) * co]
        return out

    def pack2_dup112(W):
        # conv2: pair-packed K=128 (2 taps x 64ci); M=112 with the dup copy
        # at rows 64:112 so both PSUM write-outs sit at 32-aligned bases
        P = pack2(W, 64, 48)             # [128, 96] pairs of [128, 48]
        out = np.zeros((128, 224), np.float32)
        for g in range(2):
            out[:, g * 112:g * 112 + 48] = P[:, g * 48:(g + 1) * 48]
            out[:, g * 112 + 64:g * 112 + 112] = P[:, g * 48:(g + 1) * 48]
        return out
    p['W2ed'] = pack2_dup112(W2e)        # [128, 224] pairs of [128, 112]
    p['W2od'] = pack2_dup112(W2o)

    def pack2_pad128(W):
        # conv3: pair-packed with tau'=0 at K rows 0:48 and tau'=1 at rows
        # 64:112, matching h2q's (main, replica) row placement; rows
        # 48:64/112:128 are zero (h2q is fully zeroed at startup)
        P = pack2(W, 48, 32)             # [96, 64] pairs of [96, 32]
        out = np.zeros((128, 64), np.float32)
        for g in range(2):
            out[0:48, g * 32:(g + 1) * 32] = P[0:48, g * 32:(g + 1) * 32]
            out[64:112, g * 32:(g + 1) * 32] = P[48:96, g * 32:(g + 1) * 32]
        return out
    p['W3e2'] = pack2_pad128(W3e)        # [128, 64] pairs of [128, 32]
    p['W3o2'] = pack2_pad128(W3o)
    p['cb1d'] = np.tile(f('up_b0'), 2).reshape(128, 1)
    cb2q = np.zeros((128, 1), np.float32)
    cb2q[0:48, 0] = f('up_b1')
    cb2q[64:112, 0] = f('up_b1')
    p['cb2q'] = cb2q
    p['cb3'] = f('up_b2').reshape(32, 1)
    ow = f('out_w')[0]                    # (32, 11)
    Wog = np.zeros((128, 3), np.float32)
    for g in range(3):
        for r in range(4):
            t = 4 * g + r
            if t < 11:
                Wog[32 * r:32 * r + 32, g] = ow[:, t]
    p['Wog'] = Wog
    p['ob_half'] = float(f('out_b')[0]) / 2.0
    p['obh128'] = np.full((128, 1), p['ob_half'], np.float32)

    # ---- pack into blobs ----
    packed = {}
    for blob, names in (('wf32', F32_PACK), ('wb16a', BF16A_PACK),
                        ('wb16b', BF16B_PACK)):
        width = sum(p[n].shape[1] for n in names)
        buf = np.zeros((128, width), np.float32)
        col = 0
        for n in names:
            a = p[n]
            buf[:a.shape[0], col:col + a.shape[1]] = a
            col += a.shape[1]
        packed[blob] = buf
    packed['shapes'] = {n: p[n].shape for n in
                        list(F32_PACK) + list(BF16A_PACK) + list(BF16B_PACK)}
    packed['ob_half'] = p['ob_half']
    packed['ln0_identity'] = bool(np.all(inp['ln0_g'] == 1.0)
                                  and np.all(inp['ln0_b'] == 0.0))
    packed['ln1_identity'] = bool(np.all(inp['ln1_g'] == 1.0)
                                  and np.all(inp['ln1_b'] == 0.0))
    packed['fb1_zero'] = bool(np.all(inp['ff_b1'] == 0.0))
    return packed


F32_PACK = ('fb0', 'cb1d', 'cb2q', 'cb3', 'obh128')
BF16A_PACK = ('Wt', 'Wx97', 'blockWa4', 'identb', 'bandmask',
              'one1_128', 'grows')
BF16B_PACK = ('w0T', 'w1T', 'W1ed', 'W1od', 'W2ed', 'W2od',
              'W3e2', 'W3o2', 'Wog')


# ----------------------------------------------------------------------------
# device kernel build
# ----------------------------------------------------------------------------

def _bcast_free(ap_full, offset_ap, counts):
    """Custom AP on the same tensor: dims [[pstep, 128]] + counts pairs."""
    pstep = ap_full.ap[0][0]
    return bass.AP(ap_full.tensor, offset_ap.offset,
                   [[pstep, ap_full.ap[0][1]]] + list(counts))


def _build(nc, tc, t_in, t_out, tp):
    x_ap = t_in.ap()          # [96, 512] fp32 in DRAM
    # banded-A scratch: 4 regions [128 rows (i), 192 cols (j window)]
    adense = nc.dram_tensor("adense", [4, 128, 192], BF16)
    RPITCH = 192
    RSTRIDE = 128 * RPITCH    # 24576 elements per region

    with ExitStack() as ctx:
        pw = ctx.enter_context(tc.tile_pool(name="weights", bufs=1))
        ps = ctx.enter_context(tc.tile_pool(name="seq", bufs=1))

        shapes = tp['shapes']
        wb16a = pw.tile(list(tp['wb16a'][1]), BF16, tag="wb16a")
        wf32 = pw.tile(list(tp['wf32'][1]), F32, tag="wf32")
        wb16b = pw.tile(list(tp['wb16b'][1]), BF16, tag="wb16b")
        w = {}
        for blob_tile, names in ((wf32, F32_PACK), (wb16a, BF16A_PACK),
                                 (wb16b, BF16B_PACK)):
            col = 0
            for n in names:
                r, cw = shapes[n]
                w[n] = blob_tile[0:r, col:col + cw]
                col += cw

        # ---------------- stage 0: input + weight loads ----------------
        # x arrives as bf16 and lands directly in the padded Xpb tile (no
        # cast op); critical loads lead each ring, bulk blobs drain behind
        # them during q/k + the band, and the adense zero-fill rides last
        # on the scalar ring so both rings are clean again by scatter time.
        Xpb = ps.tile([97, 576], BF16, tag="Xpb")
        nc.gpsimd.memset(Xpb[0:96, 0:32], 0.0)
        nc.gpsimd.memset(Xpb[0:96, 544:576], 0.0)
        nc.gpsimd.memset(Xpb[96:97, 0:576], 1.0)
        nc.sync.dma_start(Xpb[0:96, 32:544], x_ap)
        nc.scalar.dma_start(wb16a[:], tp['wb16a'][0].ap())
        nc.sync.dma_start(wf32[:], tp['wf32'][0].ap())
        nc.scalar.dma_start(wb16b[:], tp['wb16b'][0].ap())
        zz = ps.tile([128, 768], BF16, tag="zz")
        nc.gpsimd.memset(zz[:], 0.0)
        nc.scalar.dma_start(bass.AP(adense, 0, [[768, 128], [1, 768]]), zz[:])

        # warm the exp_and_others ACT table set (Exp anchors the set;
        # Tanh/Relu/Identity/Copy ride along, so this is the only load)
        warm = ps.tile([1, 2], F32, tag="warm")
        nc.gpsimd.memset(warm[:], 0.0)
        nc.scalar.activation(warm[:], warm[:], AF.Exp)
        nc.scalar.activation(warm[:], warm[:], AF.Tanh)
        eps128 = ps.tile([128, 1], F32, tag="eps128")
        nc.gpsimd.memset(eps128[:], EPS_LN)

        # ---------------- attention: q/k (+halo wings via matmul) -------
        Q4 = ps.tile([128, 128], BF16, tag="Q4")
        K4pad = ps.tile([128, 192], BF16, tag="K4pad")
        nc.gpsimd.memset(K4pad[96:128, 160:192], 0.0)

        with tc.tile_pool(name="qk_ps", bufs=3, space="PSUM") as pp:
            k_ps = pp.tile([128, 128], F32, tag="qk")
            for c in range(4):
                nc.tensor.matmul(k_ps[32 * c:32 * c + 32, :], w['Wx97'],
                                 Xpb[0:97, 32 + c * 128:32 + (c + 1) * 128],
                                 tile_position=(0, 32 * c))
            nc.vector.tensor_copy(K4pad[:, 32:160], k_ps[:])
            # halo wings: left = k of previous chunk's last 32 cols,
            # right = k of next chunk's first 32 cols
            wing_ps = pp.tile([128, 64], F32, tag="qk")
            for c in range(4):
                nc.tensor.matmul(wing_ps[32 * c:32 * c + 32, 0:32], w['Wx97'],
                                 Xpb[0:97, c * 128:c * 128 + 32],
                                 tile_position=(0, 32 * c))
            for c in range(3):
                nc.tensor.matmul(wing_ps[32 * c:32 * c + 32, 32:64], w['Wx97'],
                                 Xpb[0:97, 32 + (c + 1) * 128:
                                     64 + (c + 1) * 128],
                                 tile_position=(0, 32 * c))
            nc.scalar.copy(K4pad[:, 0:32], wing_ps[:, 0:32])
            nc.scalar.copy(K4pad[0:96, 160:192], wing_ps[0:96, 32:64])
            q_ps = pp.tile([128, 128], F32, tag="qk")
            for c in range(4):
                nc.tensor.matmul(q_ps[32 * c:32 * c + 32, :], w['Wt'],
                                 Xpb[0:96, 32 + c * 128:32 + (c + 1) * 128],
                                 tile_position=(0, 32 * c))
            nc.vector.tensor_copy(Q4[:], q_ps[:])

        # LN gamma/beta (+ffn b1) broadcast rows; skipped entirely when the
        # affine is identity and ff_b1 is zero (host-checked)
        need_gb = not (tp['ln0_identity'] and tp['ln1_identity']
                       and tp['fb1_zero'])
        if need_gb:
            GBb = ps.tile([128, 480], BF16, tag="GBb")
            with tc.tile_pool(name="gb_ps", bufs=1, space="PSUM") as gbp:
                gb_ps = gbp.tile([128, 480], F32, tag="gb")
                nc.tensor.matmul(gb_ps[:], w['one1_128'], w['grows'])
                nc.vector.tensor_copy(GBb[:], gb_ps[:])
            Gb0, Bb0 = GBb[:, 0:96], GBb[:, 96:192]
            Gb1, Bb1 = GBb[:, 192:288], GBb[:, 288:384]
            Fb = GBb[:, 384:480]
        else:
            Gb0 = Bb0 = Gb1 = Bb1 = Fb = None

        # ---------------- attention: band logits ----------------
        # The Xw/XiT window transposes for AV are interleaved into the band:
        # group g's add/tanh/E-matmuls are followed by chunk g's transposes
        # (PE idle gaps) and copies (DVE slack behind the serial tanh chain).
        GO = 16  # offsets per group
        EXb = ps.tile([128, 256], BF16, tag="EXb")
        EXf = ps.tile([128, 256], BF16, tag="EXf")
        Xw = []
        XiT = ps.tile([128, 384], BF16, tag="XiT")

        with ExitStack() as ectx:
            pa_arg = ectx.enter_context(tc.tile_pool(name="arg_sb", bufs=2))
            pa_tan = ectx.enter_context(tc.tile_pool(name="tan_sb", bufs=3))
            pe = ectx.enter_context(tc.tile_pool(name="e_ps", bufs=1,
                                                 space="PSUM"))
            xt = ectx.enter_context(tc.tile_pool(name="xw_ps", bufs=2,
                                                 space="PSUM"))
            xt2 = ectx.enter_context(tc.tile_pool(name="xi_ps", bufs=2,
                                                  space="PSUM"))
            E_ps = pe.tile([128, 256], F32, tag="E")
            # first group halved so the serial tanh chain starts ~0.6us
            # earlier (the add is the only thing ahead of it)
            GROUPS = ((0, 8), (8, 8), (16, 16), (32, 16), (48, 16))
            for g, (o0, go) in enumerate(GROUPS):
                Targ = pa_arg.tile([128, go * 128], BF16, tag=f"Targ{go}")
                q_b = _bcast_free(Q4[:], Q4[:], [[0, go], [1, 128]])
                k_b = _bcast_free(K4pad[:], K4pad[:, o0:192], [[1, go], [1, 128]])
                nc.vector.tensor_add(
                    Targ[:].rearrange("p (o i) -> p o i", o=go), q_b, k_b)
                Ttan = pa_tan.tile([128, go * 128], BF16, tag=f"Ttan{go}")
                nc.scalar.activation(Ttan[:], Targ[:], AF.Tanh)
                for oi in range(go):
                    o = o0 + oi
                    nc.tensor.matmul(
                        E_ps[:].rearrange("p (c o) -> p c o", o=64)[:, :, o],
                        Ttan[:, oi * 128:(oi + 1) * 128], w['blockWa4'])
                if g == 2:
                    # o 0:32 of every chunk is complete: exp+mask+scatter
                    # the first half of the band under the remaining tanhs
                    # (rings and DVE are idle; one 400ns wedge on Scalar)
                    h1 = EXf[:].rearrange("p (c o) -> p c o", o=64)[:, :, 0:32]
                    nc.scalar.activation(
                        h1, E_ps[:].rearrange("p (c o) -> p c o", o=64)[:, :, 0:32],
                        AF.Exp)
                    mh1 = EXb[:].rearrange("p (c o) -> p c o", o=64)[:, :, 0:32]
                    nc.vector.tensor_mul(
                        mh1, h1,
                        w['bandmask'].rearrange("p (c o) -> p c o", o=64)[:, :, 0:32])
                    exb_ap = EXb[:]
                    nc.sync.dma_start(
                        bass.AP(adense, 0,
                                [[RPITCH + 1, 128], [2 * RSTRIDE, 2], [1, 32]]),
                        bass.AP(exb_ap.tensor, exb_ap.offset,
                                [[256, 128], [128, 2], [1, 32]]))
                    nc.scalar.dma_start(
                        bass.AP(adense, RSTRIDE,
                                [[RPITCH + 1, 128], [2 * RSTRIDE, 2], [1, 32]]),
                        bass.AP(exb_ap.tensor, exb_ap.offset + 64,
                                [[256, 128], [128, 2], [1, 32]]))
                if g == 0:
                    continue
                c = g - 1
                # Xw windows split as [j_loc 64:192] (cols 0:96, full 128
                # rows) + [j_loc 0:64] (cols 96:192, rows 0:64) to match the
                # XBAR-transposed At pieces
                x_ps = xt.tile([128, 192], BF16, tag="x")
                nc.tensor.transpose(x_ps[:, 0:96],
                                    Xpb[0:96, c * 128 + 64:c * 128 + 192],
                                    w['identb'][0:96, 0:96])
                nc.tensor.transpose(x_ps[0:64, 96:192],
                                    Xpb[0:96, c * 128:c * 128 + 64],
                                    w['identb'][0:96, 0:96])
                xw = ps.tile([128, 192], BF16, tag=f"Xw{c}")
                nc.vector.tensor_copy(xw[:, 0:96], x_ps[:, 0:96])
                nc.vector.tensor_copy(xw[0:64, 96:192], x_ps[0:64, 96:192])
                Xw.append(xw)
                xi_ps = xt2.tile([128, 96], BF16, tag="xi")
                nc.tensor.transpose(xi_ps[:],
                                    Xpb[0:96, 32 + c * 128:32 + (c + 1) * 128],
                                    w['identb'][0:96, 0:96])
                nc.vector.tensor_copy(XiT[:, c * 96:(c + 1) * 96], xi_ps[:])
                if g == 1:
                    # conv tile pads (consumed from LN1 onwards)
                    h0 = ps.tile([96, 516], BF16, tag="h0")
                    nc.gpsimd.memset(h0[:, 0:2], 0.0)
                    nc.gpsimd.memset(h0[:, 514:516], 0.0)
                    h1rep = ps.tile([128, 1028], BF16, tag="h1rep")
                    nc.gpsimd.memset(h1rep[:, 0:2], 0.0)
                    nc.gpsimd.memset(h1rep[:, 1024:1028], 0.0)
                    h2q = ps.tile([128, 2052], BF16, tag="h2q")
                    nc.gpsimd.memset(h2q[:], 0.0)
                    h3rep = ps.tile([128, 4112], BF16, tag="h3rep")
                    nc.gpsimd.memset(h3rep[:, 0:8], 0.0)
                    nc.gpsimd.memset(h3rep[:, 4104:4112], 0.0)
            # masked unnormalized weights, bf16 end to end (second half;
            # the first half went out mid-band)
            h2 = EXf[:].rearrange("p (c o) -> p c o", o=64)[:, :, 32:64]
            nc.scalar.activation(
                h2, E_ps[:].rearrange("p (c o) -> p c o", o=64)[:, :, 32:64],
                AF.Exp)
            nc.vector.tensor_mul(
                EXb[:].rearrange("p (c o) -> p c o", o=64)[:, :, 32:64], h2,
                w['bandmask'].rearrange("p (c o) -> p c o", o=64)[:, :, 32:64])


        exb_ap = EXb[:]
        nc.sync.dma_start(
            bass.AP(adense, 32,
                    [[RPITCH + 1, 128], [2 * RSTRIDE, 2], [1, 32]]),
            bass.AP(exb_ap.tensor, exb_ap.offset + 32,
                    [[256, 128], [128, 2], [1, 32]]))
        nc.scalar.dma_start(
            bass.AP(adense, RSTRIDE + 32,
                    [[RPITCH + 1, 128], [2 * RSTRIDE, 2], [1, 32]]),
            bass.AP(exb_ap.tensor, exb_ap.offset + 96,
                    [[256, 128], [128, 2], [1, 32]]))
        S4 = ps.tile([128, 4], F32, tag="S4")
        nc.vector.tensor_reduce(S4[:], EXb[:].rearrange("p (c o) -> p c o", o=64),
                                AX.X, ALU.add)
        nc.vector.tensor_scalar_add(S4[:], S4[:], EPS_ATTN)
        R4 = ps.tile([128, 4], F32, tag="R4")
        nc.vector.reciprocal(R4[:], S4[:])
        # pull the rsqrt table-set load into the scatter/gather latency
        # window; reading R4 anchors it after the softmax exp + row sums
        # (the scheduler hoists dep-free ops arbitrarily early)
        warmr = ps.tile([1, 2], F32, tag="warmr")
        nc.scalar.activation(warmr[:], R4[0:1, 0:2], AF.Abs_reciprocal_sqrt)

        # ---------------- attention: AV (v^T[i,ch]) + LN0 stats ----------
        vT_i = ps.tile([128, 384], BF16, tag="vTi")
        bns0 = ps.tile([128, 24], F32, tag="bns0")
        MV0 = ps.tile([128, 8], F32, tag="MV0")
        with ExitStack() as actx:
            pad = actx.enter_context(tc.tile_pool(name="ad_sb", bufs=4))
            pat = actx.enter_context(tc.tile_pool(name="at_sb", bufs=4))
            ptp = actx.enter_context(tc.tile_pool(name="at_ps", bufs=3,
                                                  space="PSUM"))
            pv = actx.enter_context(tc.tile_pool(name="v_ps", bufs=2,
                                                 space="PSUM"))
            for c in range(4):
                Ad = pad.tile([128, 192], BF16, tag="Ad")
                eng = nc.sync if c % 2 == 0 else nc.scalar
                eng.dma_start(Ad[:], bass.AP(adense, c * RSTRIDE,
                                             [[RPITCH, 128], [1, RPITCH]]))
                t_ps = ptp.tile([128, 256], BF16, tag="tp")
                nc.tensor.transpose(t_ps[:, 0:128], Ad[:, 64:192],
                                    w['identb'])
                nc.tensor.transpose(t_ps[0:64, 128:256], Ad[:, 0:64],
                                    w['identb'])
                At = pat.tile([128, 256], BF16, tag="At")
                nc.vector.tensor_copy(At[:, 0:128], t_ps[:, 0:128])
                nc.scalar.copy(At[0:64, 128:256], t_ps[0:64, 128:256])
                v_ps = pv.tile([128, 96], F32, tag="v")
                nc.tensor.matmul(v_ps[:], At[:, 0:128], Xw[c][:, 0:96],
                                 start=True, stop=False)
                nc.tensor.matmul(v_ps[:], At[0:64, 128:256],
                                 Xw[c][0:64, 96:192],
                                 start=False, stop=True)
                # v*R + x^T in one pass (R4 is per-partition here)
                nc.vector.scalar_tensor_tensor(
                    vT_i[:, c * 96:(c + 1) * 96], v_ps[:], R4[:, c:c + 1],
                    XiT[:, c * 96:(c + 1) * 96], ALU.mult, ALU.add)
                nc.vector.bn_stats(bns0[:, 6 * c:6 * c + 6],
                                   vT_i[:, c * 96:(c + 1) * 96])
                nc.vector.bn_aggr(MV0[:, 2 * c:2 * c + 2],
                                  bns0[:, 6 * c:6 * c + 6])

        # ---------------- LN tails (i-layout, ACT rsqrt) ----------------
        def rstd_act(MV, tag):
            """rstd[128,4] = Rsqrt(var+eps) in one ACT op; the
            reciprocal_sqrt_and_small table set is pre-warmed during the
            attention scatter/gather window and also covers the FFN/conv
            Relus, so no load lands on the LN critical path."""
            rstd = ps.tile([128, 4], F32, tag=f"rstd{tag}", name=f"rstd{tag}")
            mv_ap = MV[:]
            var_ap = bass.AP(mv_ap.tensor, mv_ap.offset + 1, [[8, 128], [2, 4]])
            nc.scalar.activation(rstd[:], var_ap, AF.Abs_reciprocal_sqrt,
                                 bias=eps128[:])
            return rstd

        rstd_aps = {}

        def ln_i(MV, src, Gb, Bb, identity, sink, tag, keep=False):
            rstd = rstd_act(MV, tag)
            rstd_aps[tag] = rstd
            with tc.tile_pool(name=f"ln{tag}_sb", bufs=2) as ly:
                for c in range(4):
                    def final_tile():
                        if keep:
                            return ps.tile([128, 96], BF16, tag=f"yk{tag}{c}",
                                           name=f"yk{tag}{c}")
                        return ly.tile([128, 96], BF16, tag="yf",
                                       name=f"yf{tag}")
                    if identity:
                        y0 = final_tile()
                    else:
                        y0 = ly.tile([128, 96], BF16, tag="y0",
                                     name=f"y0{tag}")
                    nc.vector.tensor_scalar(y0[:], src[:, c * 96:(c + 1) * 96],
                                            MV[:, 2 * c:2 * c + 1],
                                            rstd[:, c:c + 1],
                                            ALU.subtract, ALU.mult)
                    if identity:
                        sink(c, y0)
                        continue
                    y1 = ly.tile([128, 96], BF16, tag="y1", name=f"y1{tag}")
                    nc.vector.tensor_mul(y1[:], y0[:], Gb)
                    y2 = final_tile()
                    nc.vector.tensor_add(y2[:], y1[:], Bb)
                    sink(c, y2)

        # LN0 output chunks stay live (x2 residual for FFN-2) - no i-layout
        # copy needed; x2b is the transposed view for the FFN-1 rhs
        x2c = []
        x2b = ps.tile([96, 512], BF16, tag="x2b")
        with tc.tile_pool(name="ln0_ps", bufs=2, space="PSUM") as lp0:
            def sink0(c, y2):
                x2c.append(y2)
                xp_ps = lp0.tile([96, 128], BF16, tag="xp", name="xp0")
                nc.tensor.transpose(xp_ps[:], y2[:], w['identb'])
                nc.scalar.copy(x2b[:, c * 128:(c + 1) * 128], xp_ps[:])
            ln_i(MV0, vT_i, Gb0, Bb0, tp['ln0_identity'], sink0, "0",
                 keep=True)

        # ---------------- FFN (chunk-pipelined) ----------------
        x4T = ps.tile([128, 384], BF16, tag="x4T")
        bns1 = ps.tile([128, 24], F32, tag="bns1")
        MV1 = ps.tile([128, 8], F32, tag="MV1")
        with ExitStack() as fctx:
            fp = fctx.enter_context(tc.tile_pool(name="ffn_sb", bufs=1))
            fpp = fctx.enter_context(tc.tile_pool(name="ffn_ps", bufs=1,
                                                  space="PSUM"))
            f2 = fctx.enter_context(tc.tile_pool(name="f2_sb", bufs=2))
            f2p = fctx.enter_context(tc.tile_pool(name="f2_ps", bufs=2,
                                                  space="PSUM"))
            h_ps = [fpp.tile([128, 512], F32, tag=f"h{s}", name=f"h{s}")
                    for s in range(3)]
            Hr = [fp.tile([128, 512], BF16, tag=f"hr{s}", name=f"hr{s}")
                  for s in range(3)]
            for c in range(4):
                sl = slice(c * 128, (c + 1) * 128)
                for s in range(3):
                    nc.tensor.matmul(h_ps[s][:, sl],
                                     w['w0T'][:, s * 128:(s + 1) * 128],
                                     x2b[:, sl])
                    if (c + s) % 2 == 0:
                        nc.scalar.activation(Hr[s][:, sl], h_ps[s][:, sl],
                                             AF.Relu, bias=w['fb0'][:, s:s + 1])
                    else:
                        nc.vector.tensor_scalar(Hr[s][:, sl], h_ps[s][:, sl],
                                                w['fb0'][:, s:s + 1], 0.0,
                                                ALU.add, ALU.max)
                # FFN-2 transposed: x3^T[i, ch] for this chunk
                x3_ps = f2p.tile([128, 96], F32, tag="x3T", name="x3T")
                for s in range(3):
                    nc.tensor.matmul(x3_ps[:], Hr[s][:, sl],
                                     w['w1T'][:, s * 96:(s + 1) * 96],
                                     start=(s == 0), stop=(s == 2))
                if tp['fb1_zero']:
                    nc.vector.tensor_add(x4T[:, c * 96:(c + 1) * 96],
                                         x3_ps[:], x2c[c][:])
                else:
                    t0 = f2.tile([128, 96], BF16, tag="t0", name="t0")
                    nc.vector.tensor_add(t0[:], x3_ps[:], Fb)
                    nc.vector.tensor_add(x4T[:, c * 96:(c + 1) * 96], t0[:],
                                         x2c[c][:])
                nc.vector.bn_stats(bns1[:, 6 * c:6 * c + 6],
                                   x4T[:, c * 96:(c + 1) * 96])
                nc.vector.bn_aggr(MV1[:, 2 * c:2 * c + 2],
                                  bns1[:, 6 * c:6 * c + 6])

        # ---------------- LN1 (i-layout) -> h0 ----------------
        with tc.tile_pool(name="ln1_ps", bufs=2, space="PSUM") as lp1:
            def sink1(c, y2):
                xp_ps = lp1.tile([96, 128], BF16, tag="xp", name="xp1")
                nc.tensor.transpose(xp_ps[:], y2[:], w['identb'])
                if c % 2 == 0:
                    nc.vector.tensor_copy(h0[:, 2 + c * 128:2 + (c + 1) * 128],
                                          xp_ps[:])
                else:
                    nc.scalar.copy(h0[:, 2 + c * 128:2 + (c + 1) * 128],
                                   xp_ps[:])
            ln_i(MV1, x4T, Gb1, Bb1, tp['ln1_identity'], sink1, "1")

        # ---------------- conv stack ----------------
        def strided_dst(hout, p0, cout, col0, n=512):
            hap = hout[:]
            return bass.AP(hap.tensor, hout[p0:p0 + 1, col0:col0 + 1].offset,
                           [[hap.ap[0][0], cout], [2, n]])

        # epilogue column pieces: the first piece covers everything the next
        # conv's k=0 matmuls read (dst cols <= 516+par), so they can start
        # while the second piece is still being written
        EPI_SPLIT = ((0, 258), (258, 254))

        # conv1: K=96, 4 taps x 2 phases, M=128 (dup) -> h1 + shifted replica
        with tc.tile_pool(name="c1_ps", bufs=1, space="PSUM") as cp1:
            zs1 = {}
            for par, Wp in ((0, 'W1ed'), (1, 'W1od')):
                z = cp1.tile([128, 512], F32, tag=f"z1{par}", name=f"z1{par}")
                zs1[par] = z
                for tau in range(4):
                    nc.tensor.matmul(z[:], w[Wp][:, tau * 128:(tau + 1) * 128],
                                     h0[:, par + tau:par + tau + 512],
                                     start=(tau == 0), stop=(tau == 3))
            for par in (0, 1):
                z = zs1[par]
                if par == 0:
                    nc.scalar.activation(strided_dst(h1rep, 0, 64, 2 + par),
                                         z[0:64, :], AF.Relu,
                                         bias=w['cb1d'][0:64, :])
                    nc.vector.tensor_scalar(
                        strided_dst(h1rep, 64, 64, 1 + par), z[64:128, :],
                        w['cb1d'][64:128, :], 0.0, ALU.add, ALU.max)
                else:
                    nc.vector.tensor_scalar(
                        strided_dst(h1rep, 0, 64, 2 + par), z[0:64, :],
                        w['cb1d'][0:64, :], 0.0, ALU.add, ALU.max)
                    nc.scalar.activation(strided_dst(h1rep, 64, 64, 1 + par),
                                         z[64:128, :], AF.Relu,
                                         bias=w['cb1d'][64:128, :])

        # conv2: K=128 (2 taps packed), M=112 (dup at rows 64:112) ->
        # h2q rows 0:48 main, rows 64:112 shifted replica
        with tc.tile_pool(name="c2_ps", bufs=1, space="PSUM") as cp2:
            psums = {}
            for par in (0, 1):
                for k in range(2):
                    psums[(par, k)] = cp2.tile([112, 512], F32,
                                               tag=f"z2{par}{k}",
                                               name=f"z2_{par}_{k}")
            for par, Wp in ((0, 'W2ed'), (1, 'W2od')):
                for pq in range(2):
                    for k in range(2):
                        nc.tensor.matmul(
                            psums[(par, k)][:],
                            w[Wp][:, pq * 112:(pq + 1) * 112],
                            h1rep[:, par + 2 * pq + k * 512:
                                  par + 2 * pq + k * 512 + 512],
                            start=(pq == 0), stop=(pq == 1))
            for par in (0, 1):
                for k in range(2):
                    z = psums[(par, k)]
                    c0 = 2 + 2 * k * 512 + par
                    if (par + k) % 2 == 0:
                        nc.scalar.activation(strided_dst(h2q, 0, 48, c0),
                                             z[0:48, :], AF.Relu,
                                             bias=w['cb2q'][0:48, :])
                        nc.vector.tensor_scalar(
                            strided_dst(h2q, 64, 48, c0 - 1), z[64:112, :],
                            w['cb2q'][64:112, :], 0.0, ALU.add, ALU.max)
                    else:
                        nc.vector.tensor_scalar(
                            strided_dst(h2q, 0, 48, c0), z[0:48, :],
                            w['cb2q'][0:48, :], 0.0, ALU.add, ALU.max)
                        nc.scalar.activation(strided_dst(h2q, 64, 48, c0 - 1),
                                             z[64:112, :], AF.Relu,
                                             bias=w['cb2q'][64:112, :])

        # pull the tanh/exp table-set reload into the conv2/conv3 matmul
        # window so the sigmoid tail doesn't pay for it; reading the first
        # h2q column anchors it behind conv2's first epilogue only
        warmt = ps.tile([1, 2], BF16, tag="warmt")
        nc.scalar.activation(warmt[:], h2q[0:1, 2:4], AF.Tanh)

        # conv3: K=96 (2 taps packed), 2 pairs x 2 phases x 4 chunks;
        # 4-fold replicas for the output conv built per chunk so DMA
        # receipts hide under later chunks' compute
        with tc.tile_pool(name="c3_ps", bufs=1, space="PSUM") as cp3:
            psums = {}
            for par in (0, 1):
                for k in range(4):
                    psums[(par, k)] = cp3.tile([32, 512], F32,
                                               tag=f"z3{par}{k}",
                                               name=f"z3_{par}_{k}")
            for par, Wp in ((0, 'W3e2'), (1, 'W3o2')):
                for pq in range(2):
                    for k in range(4):
                        nc.tensor.matmul(
                            psums[(par, k)][:],
                            w[Wp][:, pq * 32:(pq + 1) * 32],
                            h2q[:, par + 2 * pq + k * 512:
                                par + 2 * pq + k * 512 + 512],
                            start=(pq == 0), stop=(pq == 1))
            for k in range(4):
                for par in (0, 1):
                    z = psums[(par, k)]
                    c0 = 8 + 2 * k * 512 + par
                    if par == 0:
                        nc.scalar.activation(strided_dst(h3rep, 0, 32, c0),
                                             z[:], AF.Relu, bias=w['cb3'])
                    else:
                        nc.vector.tensor_scalar(strided_dst(h3rep, 0, 32, c0),
                                                z[:], w['cb3'], 0.0,
                                                ALU.add, ALU.max)
                C0 = 8 + 2 * k * 512
                hi = 4112 if k == 3 else C0 + 1024
                for r in (1, 2, 3):
                    eng = (nc.sync, nc.scalar, nc.sync)[r - 1] if k % 2 == 0 \
                        else (nc.scalar, nc.sync, nc.scalar)[r - 1]
                    eng.dma_start(h3rep[32 * r:32 * r + 32, C0 - r:hi - r],
                                  h3rep[0:32, C0:hi])


        # ---------------- output conv (col-tiled) + sigmoid ----------------
        with ExitStack() as octx:
            yo = octx.enter_context(tc.tile_pool(name="yo_sb", bufs=2))
            yp = octx.enter_context(tc.tile_pool(name="yo_ps", bufs=2,
                                                 space="PSUM"))
            for b in range(2):
                y_ps = yp.tile([128, 512], F32, tag="yo", name="y_ps")
                for j in range(4):
                    k = 4 * b + j
                    for g in range(3):
                        rhs = h3rep[:, k * 512 + 4 * g + 3:
                                    k * 512 + 4 * g + 3 + 512]
                        nc.tensor.matmul(y_ps[32 * j:32 * j + 1, :],
                                         w['Wog'][:, g:g + 1], rhs,
                                         start=(g == 0), stop=(g == 2),
                                         tile_position=(0, 32 * j))
                ysig = yo.tile([128, 512], F32, tag="ysig", name="ysig")
                nc.scalar.activation(ysig[0:97, :], y_ps[0:97, :], AF.Tanh,
                                     bias=w['obh128'][0:97, :], scale=0.5)
                out4 = yo.tile([128, 512], F32, tag="out4", name="out4")
                nc.vector.tensor_scalar(out4[0:97, :], ysig[0:97, :],
                                        0.5, 0.5, ALU.mult, ALU.add)
                o_ap = out4[:]
                src = bass.AP(o_ap.tensor, o_ap.offset,
                              [[512 * 32, 4], [1, 512]])
                dst = bass.AP(t_out, b * 2048, [[512, 4], [1, 512]])
                eng = nc.sync if b == 0 else nc.scalar
                eng.dma_start(dst, src)


# ----------------------------------------------------------------------------
# public entry point
# ----------------------------------------------------------------------------

def build_module(p):
    import ml_dtypes
    nc = bacc.Bacc("TRN2", target_bir_lowering=False, debug=False)
    t_in = nc.dram_tensor("x", [C, L], BF16, kind="ExternalInput")
    t_out = nc.dram_tensor("out", [1, 4096], F32, kind="ExternalOutput")
    tp = {}
    # weight blobs ride inside the NEFF as Const tensors: the runtime DMAs
    # them to HBM at model-load time, so no staging transfer or completion
    # wait lands in the execution window
    for blob, dt, npdt in (('wf32', F32, np.float32),
                           ('wb16a', BF16, ml_dtypes.bfloat16),
                           ('wb16b', BF16, ml_dtypes.bfloat16)):
        data = np.ascontiguousarray(p[blob].astype(npdt))
        tp[blob] = (nc.inline_tensor(data, name=blob), p[blob].shape, dt)
    tp['shapes'] = p['shapes']
    tp['ob_half'] = p['ob_half']
    tp['ln0_identity'] = p['ln0_identity']
    tp['ln1_identity'] = p['ln1_identity']
    tp['fb1_zero'] = p['fb1_zero']
    with tile.TileContext(nc) as tc:
        _build(nc, tc, t_in, t_out, tp)
    nc.compile()
    return nc


def kernel(**inputs):
    # The neuron compile cache keys on the HLO signature only (it does not
    # hash the embedded bass program), so a stale entry from a different
    # kernel revision with identical I/O shapes would silently load the
    # wrong NEFF. Purge unless the cache was stamped by this exact source.
    import hashlib
    import shutil
    me = hashlib.sha256(open(__file__, 'rb').read()).hexdigest()
    for cdir in ('/root/.neuron-compile-cache', '/var/tmp/neuron-compile-cache'):
        marker = os.path.join(cdir, '.kernel_src_hash')
        try:
            if open(marker).read() == me:
                continue
        except OSError:
            pass
        shutil.rmtree(cdir, ignore_errors=True)
        try:
            os.makedirs(cdir, exist_ok=True)
            with open(marker, 'w') as fh:
                fh.write(me)
        except OSError:
            pass

    x = np.asarray(inputs['x'], np.float32)          # (8, 96, 512)
    N = x.shape[0]
    p = _host_prep(inputs)
    nc = build_module(p)
    import ml_dtypes
    in_maps = [{'x': np.ascontiguousarray(x[n].astype(ml_dtypes.bfloat16))}
               for n in range(N)]
    res = run_bass_kernel_spmd(nc, in_maps, core_ids=list(range(N)))
    global LAST_RESULTS
    LAST_RESULTS = res
    out = np.stack([res.results[n]['out'] for n in range(N)], axis=0)
    return out.astype(np.float32)


LAST_RESULTS = None


if __name__ == '__main__':
    print("kernel.py loaded OK")


# revision 58
# speedup vs baseline: 1.1197x; 1.0220x over previous
"""Trainium2 Bass kernel for nn_Decoder_16054587752897.

Decoder block: banded additive (Bahdanau) attention + LN + FFN + LN +
3x (nearest-upsample-2x + conv1d k=7 + relu) + conv1d k=11 + sigmoid.

Sharding: pure data parallel - batch N=8, one batch element per NeuronCore.

v4 structure (from v3's 72us trace):
 - DMA ring discipline: only the two critical loads (x on the sync ring,
   the small weight blob on the scalar ring) are issued before the first
   compute; the big conv-weight blob, the f32 constants and the adense
   zero-fill are issued mid-band so their descriptors never sit ahead of
   the Xpb cast / q-k matmuls on a completion semaphore.
 - bh is folded into the k-projection (ones-row appended to Xpb, bh row
   appended to Wx), so no Scalar bias pass and no early wf32 dependency.
 - single ACT table set for the whole kernel (exp_and_others covers
   Exp/Tanh/Relu/Identity/Copy); LN rstd is computed on DVE with the
   int-bitcast Newton rsqrt, so the Sqrt set is never loaded and the
   sigmoid tail needs no table switch.
 - the Xw/XiT window transposes run before the band (PE is idle there)
   with their PSUM->SBUF copies on the otherwise-idle GpSimd engine.
 - adense uses a 192-wide row pitch; the banded-A scatter is 4 per-region
   DMAs and each chunk's gather is a contiguous-row read paired on the
   same ring as its scatter.
 - FFN runs chunk-pipelined: per 128-column chunk, the three FFN-1
   matmuls + relu feed the transposed FFN-2 accumulation immediately.
 - conv1/conv2 lhsT duplicate the output channels in M so the shifted
   replica rows land straight from PSUM; conv3's 4-fold replicas for the
   k=11 output conv are chunked SBUF DMAs; the sigmoid tail processes 4
   chunks per ACT op.
"""

import os
import sys

for _p in ("/opt/trn_rl_repo",):
    if _p not in sys.path:
        sys.path.insert(0, _p)

import numpy as np
from contextlib import ExitStack

import concourse.bass as bass
import concourse.bacc as bacc
import concourse.mybir as mybir
import concourse.tile as tile
from concourse.bass_utils import run_bass_kernel_spmd

F32 = mybir.dt.float32
I32 = mybir.dt.int32
BF16 = mybir.dt.bfloat16
AF = mybir.ActivationFunctionType
ALU = mybir.AluOpType
AX = mybir.AxisListType

L = 512
C = 96
EPS_ATTN = 1e-6
EPS_LN = 1e-5
RSQRT_MAGIC = 0x5F3759DF


# ----------------------------------------------------------------------------
# host-side constant prep (weight-only transforms)
# ----------------------------------------------------------------------------

def _host_prep(inp):
    f = lambda k: np.ascontiguousarray(np.asarray(inp[k], np.float32))
    p = {}
    p['Wt'] = f('Wt')                       # [96, 32] lhsT for q
    # bh folded into k: ones-row in Xpb meets the bh row of Wx97
    p['Wx97'] = np.vstack([f('Wx'), f('bh')[None, :]])   # [97, 32]
    Wa = f('Wa')[:, 0]
    blockWa4 = np.zeros((128, 4), np.float32)
    for c in range(4):
        blockWa4[32 * c:32 * c + 32, c] = Wa
    p['blockWa4'] = blockWa4
    il = np.arange(128)[:, None, None]
    cc = np.arange(4)[None, :, None]
    oo = np.arange(64)[None, None, :]
    jj = cc * 128 + il + oo - 32
    p['bandmask'] = ((jj >= 0) & (jj < L)).astype(np.float32).reshape(128, 256)
    p['identb'] = np.eye(128, dtype=np.float32)
    # broadcast-rows blob: LN0 g|b, LN1 g|b, ffn b1 (each 96 wide)
    grows = np.zeros((1, 480), np.float32)
    grows[0, 0:96] = f('ln0_g')
    grows[0, 96:192] = f('ln0_b')
    grows[0, 192:288] = f('ln1_g')
    grows[0, 288:384] = f('ln1_b')
    grows[0, 384:480] = f('ff_b1')
    p['grows'] = grows
    p['one1_128'] = np.ones((1, 128), np.float32)
    p['w0T'] = np.ascontiguousarray(f('ff_w0').T)                # [96, 384]
    p['fb0'] = np.ascontiguousarray(f('ff_b0').reshape(3, 128).T)  # [128, 3]
    # w1T [128, 3*96]: cols s*96+c = ff_w1[c, s*128+h]
    w1 = f('ff_w1')                                              # [96, 384]
    w1T = np.zeros((128, 288), np.float32)
    for s in range(3):
        w1T[:, s * 96:(s + 1) * 96] = w1[:, s * 128:(s + 1) * 128].T
    p['w1T'] = w1T

    def eo(w):
        # w: [co, ci, 7] -> even/odd tap-summed lhsT banks [ci, 4*co]
        We = np.stack([w[:, :, 0], w[:, :, 1] + w[:, :, 2],
                       w[:, :, 3] + w[:, :, 4], w[:, :, 5] + w[:, :, 6]])
        Wo = np.stack([w[:, :, 0] + w[:, :, 1], w[:, :, 2] + w[:, :, 3],
                       w[:, :, 4] + w[:, :, 5], w[:, :, 6]])
        co, ci = w.shape[0], w.shape[1]
        pack = lambda Ws: np.ascontiguousarray(
            Ws.transpose(2, 0, 1).reshape(ci, 4 * co))
        return pack(We), pack(Wo)

    W1e, W1o = eo(f('up_w0'))             # [96, 256]
    W2e, W2o = eo(f('up_w1'))             # [64, 192]
    W3e, W3o = eo(f('up_w2'))             # [48, 128]

    def dup_m(W, ci, co):
        # [ci, 4*co] -> [ci, 4*(2co)]: each tap block duplicated in M so the
        # PSUM rows co:2co replicate rows 0:co (written as the shifted copy)
        out = np.zeros((ci, 8 * co), np.float32)
        for t in range(4):
            out[:, t * 2 * co:t * 2 * co + co] = W[:, t * co:(t + 1) * co]
            out[:, t * 2 * co + co:(t + 1) * 2 * co] = W[:, t * co:(t + 1) * co]
        return out
    p['W1ed'] = dup_m(W1e, 96, 64)   # [96, 512] taps of [96, 128]
    p['W1od'] = dup_m(W1o, 96, 64)

    def pack2(W, ci, co):
        # [ci, 4*co] tap-major -> [2*ci, 2*co]: rows tau'*ci+c_i,
        # pair p covers taps (2p, 2p+1)
        out = np.zeros((2 * ci, 2 * co), np.float32)
        for g in range(2):
            for tau in range(2):
                t = 2 * g + tau
                out[tau * ci:(tau + 1) * ci, g * co:(g + 1) * co] = \
                    W[:, t * co:(t + 1) * co]
        return out

    def pack2_dup112(W):
        # conv2: pair-packed K=128 (2 taps x 64ci); M=112 with the dup copy
        # at rows 64:112 so both PSUM write-outs sit at 32-aligned bases
        P = pack2(W, 64, 48)             # [128, 96] pairs of [128, 48]
        out = np.zeros((128, 224), np.float32)
        for g in range(2):
            out[:, g * 112:g * 112 + 48] = P[:, g * 48:(g + 1) * 48]
            out[:, g * 112 + 64:g * 112 + 112] = P[:, g * 48:(g + 1) * 48]
        return out
    p['W2ed'] = pack2_dup112(W2e)        # [128, 224] pairs of [128, 112]
    p['W2od'] = pack2_dup112(W2o)

    def pack2_pad128(W):
        # conv3: pair-packed with tau'=0 at K rows 0:48 and tau'=1 at rows
        # 64:112, matching h2q's (main, replica) row placement; rows
        # 48:64/112:128 are zero (h2q is fully zeroed at startup)
        P = pack2(W, 48, 32)             # [96, 64] pairs of [96, 32]
        out = np.zeros((128, 64), np.float32)
        for g in range(2):
            out[0:48, g * 32:(g + 1) * 32] = P[0:48, g * 32:(g + 1) * 32]
            out[64:112, g * 32:(g + 1) * 32] = P[48:96, g * 32:(g + 1) * 32]
        return out
    p['W3e2'] = pack2_pad128(W3e)        # [128, 64] pairs of [128, 32]
    p['W3o2'] = pack2_pad128(W3o)
    p['cb1d'] = np.tile(f('up_b0'), 2).reshape(128, 1)
    cb2q = np.zeros((128, 1), np.float32)
    cb2q[0:48, 0] = f('up_b1')
    cb2q[64:112, 0] = f('up_b1')
    p['cb2q'] = cb2q
    p['cb3'] = f('up_b2').reshape(32, 1)
    ow = f('out_w')[0]                    # (32, 11)
    Wog = np.zeros((128, 3), np.float32)
    for g in range(3):
        for r in range(4):
            t = 4 * g + r
            if t < 11:
                Wog[32 * r:32 * r + 32, g] = ow[:, t]
    p['Wog'] = Wog
    p['ob_half'] = float(f('out_b')[0]) / 2.0
    p['obh128'] = np.full((128, 1), p['ob_half'], np.float32)

    # ---- pack into blobs ----
    packed = {}
    for blob, names in (('wf32', F32_PACK), ('wb16a', BF16A_PACK),
                        ('wb16b', BF16B_PACK)):
        width = sum(p[n].shape[1] for n in names)
        buf = np.zeros((128, width), np.float32)
        col = 0
        for n in names:
            a = p[n]
            buf[:a.shape[0], col:col + a.shape[1]] = a
            col += a.shape[1]
        packed[blob] = buf
    packed['shapes'] = {n: p[n].shape for n in
                        list(F32_PACK) + list(BF16A_PACK) + list(BF16B_PACK)}
    packed['ob_half'] = p['ob_half']
    packed['ln0_identity'] = bool(np.all(inp['ln0_g'] == 1.0)
                                  and np.all(inp['ln0_b'] == 0.0))
    packed['ln1_identity'] = bool(np.all(inp['ln1_g'] == 1.0)
                                  and np.all(inp['ln1_b'] == 0.0))
    packed['fb1_zero'] = bool(np.all(inp['ff_b1'] == 0.0))
    return packed


F32_PACK = ('fb0', 'cb1d', 'cb2q', 'cb3', 'obh128')
BF16A_PACK = ('Wt', 'Wx97', 'blockWa4', 'identb', 'bandmask',
              'one1_128', 'grows')
BF16B_PACK = ('w0T', 'w1T', 'W1ed', 'W1od', 'W2ed', 'W2od',
              'W3e2', 'W3o2', 'Wog')


# ----------------------------------------------------------------------------
# device kernel build
# ----------------------------------------------------------------------------

def _bcast_free(ap_full, offset_ap, counts):
    """Custom AP on the same tensor: dims [[pstep, 128]] + counts pairs."""
    pstep = ap_full.ap[0][0]
    return bass.AP(ap_full.tensor, offset_ap.offset,
                   [[pstep, ap_full.ap[0][1]]] + list(counts))


def _build(nc, tc, t_in, t_out, tp):
    x_ap = t_in.ap()          # [96, 512] fp32 in DRAM
    # banded-A scratch: 4 regions [128 rows (i), 192 cols (j window)]
    adense = nc.dram_tensor("adense", [4, 128, 192], BF16)
    RPITCH = 192
    RSTRIDE = 128 * RPITCH    # 24576 elements per region

    with ExitStack() as ctx:
        pw = ctx.enter_context(tc.tile_pool(name="weights", bufs=1))
        ps = ctx.enter_context(tc.tile_pool(name="seq", bufs=1))

        shapes = tp['shapes']
        wb16a = pw.tile(list(tp['wb16a'][1]), BF16, tag="wb16a")
        wf32 = pw.tile(list(tp['wf32'][1]), F32, tag="wf32")
        wb16b = pw.tile(list(tp['wb16b'][1]), BF16, tag="wb16b")
        w = {}
        for blob_tile, names in ((wf32, F32_PACK), (wb16a, BF16A_PACK),
                                 (wb16b, BF16B_PACK)):
            col = 0
            for n in names:
                r, cw = shapes[n]
                w[n] = blob_tile[0:r, col:col + cw]
                col += cw

        # ---------------- stage 0: input + weight loads ----------------
        # x arrives as bf16 and lands directly in the padded Xpb tile (no
        # cast op); critical loads lead each ring, bulk blobs drain behind
        # them during q/k + the band, and the adense zero-fill rides last
        # on the scalar ring so both rings are clean again by scatter time.
        Xpb = ps.tile([97, 576], BF16, tag="Xpb")
        nc.gpsimd.memset(Xpb[0:96, 0:32], 0.0)
        nc.gpsimd.memset(Xpb[0:96, 544:576], 0.0)
        nc.gpsimd.memset(Xpb[96:97, 0:576], 1.0)
        nc.sync.dma_start(Xpb[0:96, 32:544], x_ap)
        nc.scalar.dma_start(wb16a[:], tp['wb16a'][0].ap())
        nc.sync.dma_start(wf32[:], tp['wf32'][0].ap())
        nc.scalar.dma_start(wb16b[:], tp['wb16b'][0].ap())
        zz = ps.tile([128, 768], BF16, tag="zz")
        nc.gpsimd.memset(zz[:], 0.0)
        nc.scalar.dma_start(bass.AP(adense, 0, [[768, 128], [1, 768]]), zz[:])

        # warm the exp_and_others ACT table set (Exp anchors the set;
        # Tanh/Relu/Identity/Copy ride along, so this is the only load)
        warm = ps.tile([1, 2], F32, tag="warm")
        nc.gpsimd.memset(warm[:], 0.0)
        nc.scalar.activation(warm[:], warm[:], AF.Exp)
        nc.scalar.activation(warm[:], warm[:], AF.Tanh)
        eps128 = ps.tile([128, 1], F32, tag="eps128")
        nc.gpsimd.memset(eps128[:], EPS_LN)

        # ---------------- attention: q/k (+halo wings via matmul) -------
        Q4 = ps.tile([128, 128], BF16, tag="Q4")
        K4pad = ps.tile([128, 192], BF16, tag="K4pad")
        nc.gpsimd.memset(K4pad[96:128, 160:192], 0.0)

        with tc.tile_pool(name="qk_ps", bufs=3, space="PSUM") as pp:
            k_ps = pp.tile([128, 128], F32, tag="qk")
            for c in range(4):
                nc.tensor.matmul(k_ps[32 * c:32 * c + 32, :], w['Wx97'],
                                 Xpb[0:97, 32 + c * 128:32 + (c + 1) * 128],
                                 tile_position=(0, 32 * c))
            nc.vector.tensor_copy(K4pad[:, 32:160], k_ps[:])
            # halo wings: left = k of previous chunk's last 32 cols,
            # right = k of next chunk's first 32 cols
            wing_ps = pp.tile([128, 64], F32, tag="qk")
            for c in range(4):
                nc.tensor.matmul(wing_ps[32 * c:32 * c + 32, 0:32], w['Wx97'],
                                 Xpb[0:97, c * 128:c * 128 + 32],
                                 tile_position=(0, 32 * c))
            for c in range(3):
                nc.tensor.matmul(wing_ps[32 * c:32 * c + 32, 32:64], w['Wx97'],
                                 Xpb[0:97, 32 + (c + 1) * 128:
                                     64 + (c + 1) * 128],
                                 tile_position=(0, 32 * c))
            nc.scalar.copy(K4pad[:, 0:32], wing_ps[:, 0:32])
            nc.scalar.copy(K4pad[0:96, 160:192], wing_ps[0:96, 32:64])
            q_ps = pp.tile([128, 128], F32, tag="qk")
            for c in range(4):
                nc.tensor.matmul(q_ps[32 * c:32 * c + 32, :], w['Wt'],
                                 Xpb[0:96, 32 + c * 128:32 + (c + 1) * 128],
                                 tile_position=(0, 32 * c))
            nc.vector.tensor_copy(Q4[:], q_ps[:])

        # LN gamma/beta (+ffn b1) broadcast rows; skipped entirely when the
        # affine is identity and ff_b1 is zero (host-checked)
        need_gb = not (tp['ln0_identity'] and tp['ln1_identity']
                       and tp['fb1_zero'])
        if need_gb:
            GBb = ps.tile([128, 480], BF16, tag="GBb")
            with tc.tile_pool(name="gb_ps", bufs=1, space="PSUM") as gbp:
                gb_ps = gbp.tile([128, 480], F32, tag="gb")
                nc.tensor.matmul(gb_ps[:], w['one1_128'], w['grows'])
                nc.vector.tensor_copy(GBb[:], gb_ps[:])
            Gb0, Bb0 = GBb[:, 0:96], GBb[:, 96:192]
            Gb1, Bb1 = GBb[:, 192:288], GBb[:, 288:384]
            Fb = GBb[:, 384:480]
        else:
            Gb0 = Bb0 = Gb1 = Bb1 = Fb = None

        # ---------------- attention: band logits ----------------
        # The Xw/XiT window transposes for AV are interleaved into the band:
        # group g's add/tanh/E-matmuls are followed by chunk g's transposes
        # (PE idle gaps) and copies (DVE slack behind the serial tanh chain).
        GO = 16  # offsets per group
        EXb = ps.tile([128, 256], BF16, tag="EXb")
        EXf = ps.tile([128, 256], BF16, tag="EXf")
        Xw = []
        XiT = ps.tile([128, 384], BF16, tag="XiT")

        with ExitStack() as ectx:
            pa_arg = ectx.enter_context(tc.tile_pool(name="arg_sb", bufs=2))
            pa_tan = ectx.enter_context(tc.tile_pool(name="tan_sb", bufs=3))
            pe = ectx.enter_context(tc.tile_pool(name="e_ps", bufs=1,
                                                 space="PSUM"))
            xt = ectx.enter_context(tc.tile_pool(name="xw_ps", bufs=2,
                                                 space="PSUM"))
            xt2 = ectx.enter_context(tc.tile_pool(name="xi_ps", bufs=2,
                                                  space="PSUM"))
            E_ps = pe.tile([128, 256], F32, tag="E")
            # first group halved so the serial tanh chain starts ~0.6us
            # earlier (the add is the only thing ahead of it)
            GROUPS = ((0, 8), (8, 8), (16, 16), (32, 16), (48, 16))
            for g, (o0, go) in enumerate(GROUPS):
                Targ = pa_arg.tile([128, go * 128], BF16, tag=f"Targ{go}")
                q_b = _bcast_free(Q4[:], Q4[:], [[0, go], [1, 128]])
                k_b = _bcast_free(K4pad[:], K4pad[:, o0:192], [[1, go], [1, 128]])
                nc.vector.tensor_add(
                    Targ[:].rearrange("p (o i) -> p o i", o=go), q_b, k_b)
                Ttan = pa_tan.tile([128, go * 128], BF16, tag=f"Ttan{go}")
                nc.scalar.activation(Ttan[:], Targ[:], AF.Tanh)
                for oi in range(go):
                    o = o0 + oi
                    nc.tensor.matmul(
                        E_ps[:].rearrange("p (c o) -> p c o", o=64)[:, :, o],
                        Ttan[:, oi * 128:(oi + 1) * 128], w['blockWa4'])
                if g == 2:
                    # o 0:32 of every chunk is complete: exp+mask+scatter
                    # the first half of the band under the remaining tanhs
                    # (rings and DVE are idle; one 400ns wedge on Scalar)
                    h1 = EXf[:].rearrange("p (c o) -> p c o", o=64)[:, :, 0:32]
                    nc.scalar.activation(
                        h1, E_ps[:].rearrange("p (c o) -> p c o", o=64)[:, :, 0:32],
                        AF.Exp)
                    mh1 = EXb[:].rearrange("p (c o) -> p c o", o=64)[:, :, 0:32]
                    nc.vector.tensor_mul(
                        mh1, h1,
                        w['bandmask'].rearrange("p (c o) -> p c o", o=64)[:, :, 0:32])
                    exb_ap = EXb[:]
                    nc.sync.dma_start(
                        bass.AP(adense, 0,
                                [[RPITCH + 1, 128], [2 * RSTRIDE, 2], [1, 32]]),
                        bass.AP(exb_ap.tensor, exb_ap.offset,
                                [[256, 128], [128, 2], [1, 32]]))
                    nc.scalar.dma_start(
                        bass.AP(adense, RSTRIDE,
                                [[RPITCH + 1, 128], [2 * RSTRIDE, 2], [1, 32]]),
                        bass.AP(exb_ap.tensor, exb_ap.offset + 64,
                                [[256, 128], [128, 2], [1, 32]]))
                if g == 0:
                    continue
                c = g - 1
                # Xw windows split as [j_loc 64:192] (cols 0:96, full 128
                # rows) + [j_loc 0:64] (cols 96:192, rows 0:64) to match the
                # XBAR-transposed At pieces
                x_ps = xt.tile([128, 192], BF16, tag="x")
                nc.tensor.transpose(x_ps[:, 0:96],
                                    Xpb[0:96, c * 128 + 64:c * 128 + 192],
                                    w['identb'][0:96, 0:96])
                nc.tensor.transpose(x_ps[0:64, 96:192],
                                    Xpb[0:96, c * 128:c * 128 + 64],
                                    w['identb'][0:96, 0:96])
                xw = ps.tile([128, 192], BF16, tag=f"Xw{c}")
                nc.vector.tensor_copy(xw[:, 0:96], x_ps[:, 0:96])
                nc.vector.tensor_copy(xw[0:64, 96:192], x_ps[0:64, 96:192])
                Xw.append(xw)
                xi_ps = xt2.tile([128, 96], BF16, tag="xi")
                nc.tensor.transpose(xi_ps[:],
                                    Xpb[0:96, 32 + c * 128:32 + (c + 1) * 128],
                                    w['identb'][0:96, 0:96])
                nc.vector.tensor_copy(XiT[:, c * 96:(c + 1) * 96], xi_ps[:])
                if g == 1:
                    # conv tile pads (consumed from LN1 onwards)
                    h0 = ps.tile([96, 516], BF16, tag="h0")
                    nc.gpsimd.memset(h0[:, 0:2], 0.0)
                    nc.gpsimd.memset(h0[:, 514:516], 0.0)
                    h1rep = ps.tile([128, 1028], BF16, tag="h1rep")
                    nc.gpsimd.memset(h1rep[:, 0:2], 0.0)
                    nc.gpsimd.memset(h1rep[:, 1024:1028], 0.0)
                    h2q = ps.tile([128, 2052], BF16, tag="h2q")
                    nc.gpsimd.memset(h2q[:], 0.0)
                    h3rep = ps.tile([128, 4112], BF16, tag="h3rep")
                    nc.gpsimd.memset(h3rep[:, 0:8], 0.0)
                    nc.gpsimd.memset(h3rep[:, 4104:4112], 0.0)
            # masked unnormalized weights, bf16 end to end (second half;
            # the first half went out mid-band)
            h2 = EXf[:].rearrange("p (c o) -> p c o", o=64)[:, :, 32:64]
            nc.scalar.activation(
                h2, E_ps[:].rearrange("p (c o) -> p c o", o=64)[:, :, 32:64],
                AF.Exp)
            nc.vector.tensor_mul(
                EXb[:].rearrange("p (c o) -> p c o", o=64)[:, :, 32:64], h2,
                w['bandmask'].rearrange("p (c o) -> p c o", o=64)[:, :, 32:64])


        # second-half scatter split per region: gather c then waits only
        # region c's 128 descriptors instead of a 256-descriptor pair
        exb_ap = EXb[:]
        for c in range(4):
            eng = nc.sync if c % 2 == 0 else nc.scalar
            eng.dma_start(
                bass.AP(adense, c * RSTRIDE + 32, [[RPITCH + 1, 128], [1, 32]]),
                bass.AP(exb_ap.tensor, exb_ap.offset + c * 64 + 32,
                        [[256, 128], [1, 32]]))
        S4 = ps.tile([128, 4], F32, tag="S4")
        nc.vector.tensor_reduce(S4[:], EXb[:].rearrange("p (c o) -> p c o", o=64),
                                AX.X, ALU.add)
        nc.vector.tensor_scalar_add(S4[:], S4[:], EPS_ATTN)
        R4 = ps.tile([128, 4], F32, tag="R4")
        nc.vector.reciprocal(R4[:], S4[:])
        # pull the rsqrt table-set load into the scatter/gather latency
        # window; reading R4 anchors it after the softmax exp + row sums
        # (the scheduler hoists dep-free ops arbitrarily early)
        warmr = ps.tile([1, 2], F32, tag="warmr")
        nc.scalar.activation(warmr[:], R4[0:1, 0:2], AF.Abs_reciprocal_sqrt)

        # ---------------- attention: AV (v^T[i,ch]) + LN0 stats ----------
        vT_i = ps.tile([128, 384], BF16, tag="vTi")
        bns0 = ps.tile([128, 24], F32, tag="bns0")
        MV0 = ps.tile([128, 8], F32, tag="MV0")
        with ExitStack() as actx:
            pad = actx.enter_context(tc.tile_pool(name="ad_sb", bufs=4))
            pat = actx.enter_context(tc.tile_pool(name="at_sb", bufs=4))
            ptp = actx.enter_context(tc.tile_pool(name="at_ps", bufs=3,
                                                  space="PSUM"))
            pv = actx.enter_context(tc.tile_pool(name="v_ps", bufs=2,
                                                 space="PSUM"))
            for c in range(4):
                Ad = pad.tile([128, 192], BF16, tag="Ad")
                eng = nc.sync if c % 2 == 0 else nc.scalar
                eng.dma_start(Ad[:], bass.AP(adense, c * RSTRIDE,
                                             [[RPITCH, 128], [1, RPITCH]]))
                t_ps = ptp.tile([128, 256], BF16, tag="tp")
                nc.tensor.transpose(t_ps[:, 0:128], Ad[:, 64:192],
                                    w['identb'])
                nc.tensor.transpose(t_ps[0:64, 128:256], Ad[:, 0:64],
                                    w['identb'])
                At = pat.tile([128, 256], BF16, tag="At")
                nc.vector.tensor_copy(At[:, 0:128], t_ps[:, 0:128])
                nc.scalar.copy(At[0:64, 128:256], t_ps[0:64, 128:256])
                v_ps = pv.tile([128, 96], F32, tag="v")
                nc.tensor.matmul(v_ps[:], At[:, 0:128], Xw[c][:, 0:96],
                                 start=True, stop=False)
                nc.tensor.matmul(v_ps[:], At[0:64, 128:256],
                                 Xw[c][0:64, 96:192],
                                 start=False, stop=True)
                # v*R + x^T in one pass (R4 is per-partition here)
                nc.vector.scalar_tensor_tensor(
                    vT_i[:, c * 96:(c + 1) * 96], v_ps[:], R4[:, c:c + 1],
                    XiT[:, c * 96:(c + 1) * 96], ALU.mult, ALU.add)
                nc.vector.bn_stats(bns0[:, 6 * c:6 * c + 6],
                                   vT_i[:, c * 96:(c + 1) * 96])
                nc.vector.bn_aggr(MV0[:, 2 * c:2 * c + 2],
                                  bns0[:, 6 * c:6 * c + 6])

        # ---------------- LN tails (i-layout, ACT rsqrt) ----------------
        def rstd_act(MV, tag):
            """rstd[128,4] = Rsqrt(var+eps) in one ACT op; the
            reciprocal_sqrt_and_small table set is pre-warmed during the
            attention scatter/gather window and also covers the FFN/conv
            Relus, so no load lands on the LN critical path."""
            rstd = ps.tile([128, 4], F32, tag=f"rstd{tag}", name=f"rstd{tag}")
            mv_ap = MV[:]
            var_ap = bass.AP(mv_ap.tensor, mv_ap.offset + 1, [[8, 128], [2, 4]])
            nc.scalar.activation(rstd[:], var_ap, AF.Abs_reciprocal_sqrt,
                                 bias=eps128[:])
            return rstd

        rstd_aps = {}

        def ln_i(MV, src, Gb, Bb, identity, sink, tag, keep=False):
            rstd = rstd_act(MV, tag)
            rstd_aps[tag] = rstd
            with tc.tile_pool(name=f"ln{tag}_sb", bufs=2) as ly:
                for c in range(4):
                    def final_tile():
                        if keep:
                            return ps.tile([128, 96], BF16, tag=f"yk{tag}{c}",
                                           name=f"yk{tag}{c}")
                        return ly.tile([128, 96], BF16, tag="yf",
                                       name=f"yf{tag}")
                    if identity:
                        y0 = final_tile()
                    else:
                        y0 = ly.tile([128, 96], BF16, tag="y0",
                                     name=f"y0{tag}")
                    nc.vector.tensor_scalar(y0[:], src[:, c * 96:(c + 1) * 96],
                                            MV[:, 2 * c:2 * c + 1],
                                            rstd[:, c:c + 1],
                                            ALU.subtract, ALU.mult)
                    if identity:
                        sink(c, y0)
                        continue
                    y1 = ly.tile([128, 96], BF16, tag="y1", name=f"y1{tag}")
                    nc.vector.tensor_mul(y1[:], y0[:], Gb)
                    y2 = final_tile()
                    nc.vector.tensor_add(y2[:], y1[:], Bb)
                    sink(c, y2)

        # LN0 output chunks stay live (x2 residual for FFN-2) - no i-layout
        # copy needed; x2b is the transposed view for the FFN-1 rhs
        x2c = []
        x2b = ps.tile([96, 512], BF16, tag="x2b")
        with tc.tile_pool(name="ln0_ps", bufs=2, space="PSUM") as lp0:
            def sink0(c, y2):
                x2c.append(y2)
                xp_ps = lp0.tile([96, 128], BF16, tag="xp", name="xp0")
                nc.tensor.transpose(xp_ps[:], y2[:], w['identb'])
                nc.scalar.copy(x2b[:, c * 128:(c + 1) * 128], xp_ps[:])
            ln_i(MV0, vT_i, Gb0, Bb0, tp['ln0_identity'], sink0, "0",
                 keep=True)

        # ---------------- FFN (chunk-pipelined) ----------------
        x4T = ps.tile([128, 384], BF16, tag="x4T")
        bns1 = ps.tile([128, 24], F32, tag="bns1")
        MV1 = ps.tile([128, 8], F32, tag="MV1")
        with ExitStack() as fctx:
            fp = fctx.enter_context(tc.tile_pool(name="ffn_sb", bufs=1))
            fpp = fctx.enter_context(tc.tile_pool(name="ffn_ps", bufs=1,
                                                  space="PSUM"))
            f2 = fctx.enter_context(tc.tile_pool(name="f2_sb", bufs=2))
            f2p = fctx.enter_context(tc.tile_pool(name="f2_ps", bufs=2,
                                                  space="PSUM"))
            h_ps = [fpp.tile([128, 512], F32, tag=f"h{s}", name=f"h{s}")
                    for s in range(3)]
            Hr = [fp.tile([128, 512], BF16, tag=f"hr{s}", name=f"hr{s}")
                  for s in range(3)]
            for c in range(4):
                sl = slice(c * 128, (c + 1) * 128)
                for s in range(3):
                    nc.tensor.matmul(h_ps[s][:, sl],
                                     w['w0T'][:, s * 128:(s + 1) * 128],
                                     x2b[:, sl])
                    if (c + s) % 2 == 0:
                        nc.scalar.activation(Hr[s][:, sl], h_ps[s][:, sl],
                                             AF.Relu, bias=w['fb0'][:, s:s + 1])
                    else:
                        nc.vector.tensor_scalar(Hr[s][:, sl], h_ps[s][:, sl],
                                                w['fb0'][:, s:s + 1], 0.0,
                                                ALU.add, ALU.max)
                # FFN-2 transposed: x3^T[i, ch] for this chunk
                x3_ps = f2p.tile([128, 96], F32, tag="x3T", name="x3T")
                for s in range(3):
                    nc.tensor.matmul(x3_ps[:], Hr[s][:, sl],
                                     w['w1T'][:, s * 96:(s + 1) * 96],
                                     start=(s == 0), stop=(s == 2))
                if tp['fb1_zero']:
                    nc.vector.tensor_add(x4T[:, c * 96:(c + 1) * 96],
                                         x3_ps[:], x2c[c][:])
                else:
                    t0 = f2.tile([128, 96], BF16, tag="t0", name="t0")
                    nc.vector.tensor_add(t0[:], x3_ps[:], Fb)
                    nc.vector.tensor_add(x4T[:, c * 96:(c + 1) * 96], t0[:],
                                         x2c[c][:])
                nc.vector.bn_stats(bns1[:, 6 * c:6 * c + 6],
                                   x4T[:, c * 96:(c + 1) * 96])
                nc.vector.bn_aggr(MV1[:, 2 * c:2 * c + 2],
                                  bns1[:, 6 * c:6 * c + 6])

        # ---------------- LN1 (i-layout) -> h0 ----------------
        with tc.tile_pool(name="ln1_ps", bufs=2, space="PSUM") as lp1:
            def sink1(c, y2):
                xp_ps = lp1.tile([96, 128], BF16, tag="xp", name="xp1")
                nc.tensor.transpose(xp_ps[:], y2[:], w['identb'])
                if c % 2 == 0:
                    nc.vector.tensor_copy(h0[:, 2 + c * 128:2 + (c + 1) * 128],
                                          xp_ps[:])
                else:
                    nc.scalar.copy(h0[:, 2 + c * 128:2 + (c + 1) * 128],
                                   xp_ps[:])
            ln_i(MV1, x4T, Gb1, Bb1, tp['ln1_identity'], sink1, "1")

        # ---------------- conv stack ----------------
        def strided_dst(hout, p0, cout, col0, n=512):
            hap = hout[:]
            return bass.AP(hap.tensor, hout[p0:p0 + 1, col0:col0 + 1].offset,
                           [[hap.ap[0][0], cout], [2, n]])

        # epilogue column pieces: the first piece covers everything the next
        # conv's k=0 matmuls read (dst cols <= 516+par), so they can start
        # while the second piece is still being written
        EPI_SPLIT = ((0, 258), (258, 254))

        # conv1: K=96, 4 taps x 2 phases, M=128 (dup) -> h1 + shifted replica
        with tc.tile_pool(name="c1_ps", bufs=1, space="PSUM") as cp1:
            zs1 = {}
            for par, Wp in ((0, 'W1ed'), (1, 'W1od')):
                z = cp1.tile([128, 512], F32, tag=f"z1{par}", name=f"z1{par}")
                zs1[par] = z
                for tau in range(4):
                    nc.tensor.matmul(z[:], w[Wp][:, tau * 128:(tau + 1) * 128],
                                     h0[:, par + tau:par + tau + 512],
                                     start=(tau == 0), stop=(tau == 3))
            for par in (0, 1):
                z = zs1[par]
                if par == 0:
                    nc.scalar.activation(strided_dst(h1rep, 0, 64, 2 + par),
                                         z[0:64, :], AF.Relu,
                                         bias=w['cb1d'][0:64, :])
                    nc.vector.tensor_scalar(
                        strided_dst(h1rep, 64, 64, 1 + par), z[64:128, :],
                        w['cb1d'][64:128, :], 0.0, ALU.add, ALU.max)
                else:
                    nc.vector.tensor_scalar(
                        strided_dst(h1rep, 0, 64, 2 + par), z[0:64, :],
                        w['cb1d'][0:64, :], 0.0, ALU.add, ALU.max)
                    nc.scalar.activation(strided_dst(h1rep, 64, 64, 1 + par),
                                         z[64:128, :], AF.Relu,
                                         bias=w['cb1d'][64:128, :])

        # conv2: K=128 (2 taps packed), M=112 (dup at rows 64:112) ->
        # h2q rows 0:48 main, rows 64:112 shifted replica
        with tc.tile_pool(name="c2_ps", bufs=1, space="PSUM") as cp2:
            psums = {}
            for par in (0, 1):
                for k in range(2):
                    psums[(par, k)] = cp2.tile([112, 512], F32,
                                               tag=f"z2{par}{k}",
                                               name=f"z2_{par}_{k}")
            for par, Wp in ((0, 'W2ed'), (1, 'W2od')):
                for pq in range(2):
                    for k in range(2):
                        nc.tensor.matmul(
                            psums[(par, k)][:],
                            w[Wp][:, pq * 112:(pq + 1) * 112],
                            h1rep[:, par + 2 * pq + k * 512:
                                  par + 2 * pq + k * 512 + 512],
                            start=(pq == 0), stop=(pq == 1))
            for par in (0, 1):
                for k in range(2):
                    z = psums[(par, k)]
                    c0 = 2 + 2 * k * 512 + par
                    if (par + k) % 2 == 0:
                        nc.scalar.activation(strided_dst(h2q, 0, 48, c0),
                                             z[0:48, :], AF.Relu,
                                             bias=w['cb2q'][0:48, :])
                        nc.vector.tensor_scalar(
                            strided_dst(h2q, 64, 48, c0 - 1), z[64:112, :],
                            w['cb2q'][64:112, :], 0.0, ALU.add, ALU.max)
                    else:
                        nc.vector.tensor_scalar(
                            strided_dst(h2q, 0, 48, c0), z[0:48, :],
                            w['cb2q'][0:48, :], 0.0, ALU.add, ALU.max)
                        nc.scalar.activation(strided_dst(h2q, 64, 48, c0 - 1),
                                             z[64:112, :], AF.Relu,
                                             bias=w['cb2q'][64:112, :])

        # pull the tanh/exp table-set reload into the conv2/conv3 matmul
        # window so the sigmoid tail doesn't pay for it; reading the first
        # h2q column anchors it behind conv2's first epilogue only
        warmt = ps.tile([1, 2], BF16, tag="warmt")
        nc.scalar.activation(warmt[:], h2q[0:1, 2:4], AF.Tanh)

        # conv3: K=96 (2 taps packed), 2 pairs x 2 phases x 4 chunks;
        # 4-fold replicas for the output conv built per chunk so DMA
        # receipts hide under later chunks' compute
        with tc.tile_pool(name="c3_ps", bufs=1, space="PSUM") as cp3:
            psums = {}
            for par in (0, 1):
                for k in range(4):
                    psums[(par, k)] = cp3.tile([32, 512], F32,
                                               tag=f"z3{par}{k}",
                                               name=f"z3_{par}_{k}")
            for par, Wp in ((0, 'W3e2'), (1, 'W3o2')):
                for pq in range(2):
                    for k in range(4):
                        nc.tensor.matmul(
                            psums[(par, k)][:],
                            w[Wp][:, pq * 32:(pq + 1) * 32],
                            h2q[:, par + 2 * pq + k * 512:
                                par + 2 * pq + k * 512 + 512],
                            start=(pq == 0), stop=(pq == 1))
            for k in range(4):
                for par in (0, 1):
                    z = psums[(par, k)]
                    c0 = 8 + 2 * k * 512 + par
                    if par == 0:
                        nc.scalar.activation(strided_dst(h3rep, 0, 32, c0),
                                             z[:], AF.Relu, bias=w['cb3'])
                    else:
                        nc.vector.tensor_scalar(strided_dst(h3rep, 0, 32, c0),
                                                z[:], w['cb3'], 0.0,
                                                ALU.add, ALU.max)
                C0 = 8 + 2 * k * 512
                hi = 4112 if k == 3 else C0 + 1024
                for r in (1, 2, 3):
                    eng = (nc.sync, nc.scalar, nc.sync)[r - 1] if k % 2 == 0 \
                        else (nc.scalar, nc.sync, nc.scalar)[r - 1]
                    eng.dma_start(h3rep[32 * r:32 * r + 32, C0 - r:hi - r],
                                  h3rep[0:32, C0:hi])


        # ---------------- output conv (col-tiled) + sigmoid ----------------
        with ExitStack() as octx:
            yo = octx.enter_context(tc.tile_pool(name="yo_sb", bufs=2))
            yp = octx.enter_context(tc.tile_pool(name="yo_ps", bufs=2,
                                                 space="PSUM"))
            for b in range(2):
                y_ps = yp.tile([128, 512], F32, tag="yo", name="y_ps")
                for j in range(4):
                    k = 4 * b + j
                    for g in range(3):
                        rhs = h3rep[:, k * 512 + 4 * g + 3:
                                    k * 512 + 4 * g + 3 + 512]
                        nc.tensor.matmul(y_ps[32 * j:32 * j + 1, :],
                                         w['Wog'][:, g:g + 1], rhs,
                                         start=(g == 0), stop=(g == 2),
                                         tile_position=(0, 32 * j))
                ysig = yo.tile([128, 512], F32, tag="ysig", name="ysig")
                nc.scalar.activation(ysig[0:97, :], y_ps[0:97, :], AF.Tanh,
                                     bias=w['obh128'][0:97, :], scale=0.5)
                out4 = yo.tile([128, 512], F32, tag="out4", name="out4")
                nc.vector.tensor_scalar(out4[0:97, :], ysig[0:97, :],
                                        0.5, 0.5, ALU.mult, ALU.add)
                o_ap = out4[:]
                src = bass.AP(o_ap.tensor, o_ap.offset,
                              [[512 * 32, 4], [1, 512]])
                dst = bass.AP(t_out, b * 2048, [[512, 4], [1, 512]])
                eng = nc.sync if b == 0 else nc.scalar
                eng.dma_start(dst, src)


# ----------------------------------------------------------------------------
# public entry point
# ----------------------------------------------------------------------------

def build_module(p):
    import ml_dtypes
    nc = bacc.Bacc("TRN2", target_bir_lowering=False, debug=False)
    t_in = nc.dram_tensor("x", [C, L], BF16, kind="ExternalInput")
    t_out = nc.dram_tensor("out", [1, 4096], F32, kind="ExternalOutput")
    tp = {}
    # weight blobs ride inside the NEFF as Const tensors: the runtime DMAs
    # them to HBM at model-load time, so no staging transfer or completion
    # wait lands in the execution window
    for blob, dt, npdt in (('wf32', F32, np.float32),
                           ('wb16a', BF16, ml_dtypes.bfloat16),
                           ('wb16b', BF16, ml_dtypes.bfloat16)):
        data = np.ascontiguousarray(p[blob].astype(npdt))
        tp[blob] = (nc.inline_tensor(data, name=blob), p[blob].shape, dt)
    tp['shapes'] = p['shapes']
    tp['ob_half'] = p['ob_half']
    tp['ln0_identity'] = p['ln0_identity']
    tp['ln1_identity'] = p['ln1_identity']
    tp['fb1_zero'] = p['fb1_zero']
    with tile.TileContext(nc) as tc:
        _build(nc, tc, t_in, t_out, tp)
    nc.compile()
    return nc


def kernel(**inputs):
    # The neuron compile cache keys on the HLO signature only (it does not
    # hash the embedded bass program), so a stale entry from a different
    # kernel revision with identical I/O shapes would silently load the
    # wrong NEFF. Purge unless the cache was stamped by this exact source.
    import hashlib
    import shutil
    me = hashlib.sha256(open(__file__, 'rb').read()).hexdigest()
    for cdir in ('/root/.neuron-compile-cache', '/var/tmp/neuron-compile-cache'):
        marker = os.path.join(cdir, '.kernel_src_hash')
        try:
            if open(marker).read() == me:
                continue
        except OSError:
            pass
        shutil.rmtree(cdir, ignore_errors=True)
        try:
            os.makedirs(cdir, exist_ok=True)
            with open(marker, 'w') as fh:
                fh.write(me)
        except OSError:
            pass

    x = np.asarray(inputs['x'], np.float32)          # (8, 96, 512)
    N = x.shape[0]
    p = _host_prep(inputs)
    nc = build_module(p)
    import ml_dtypes
    in_maps = [{'x': np.ascontiguousarray(x[n].astype(ml_dtypes.bfloat16))}
               for n in range(N)]
    res = run_bass_kernel_spmd(nc, in_maps, core_ids=list(range(N)))
    global LAST_RESULTS
    LAST_RESULTS = res
    out = np.stack([res.results[n]['out'] for n in range(N)], axis=0)
    return out.astype(np.float32)


LAST_RESULTS = None


if __name__ == '__main__':
    print("kernel.py loaded OK")
